# revision 12
# baseline (speedup 1.0000x reference)
"""Trainium2 Bass kernel: MultiHeadCrossAttention (B=4, T=2048, C=64, H=6, D=64).

Sharding: 8 cores = 4 batches x 2 head-groups of 3. Every core runs an
identical (SPMD) program: causal flash-attention for 3 heads over the full
T=2048 sequence of one batch, followed by its partial output projection.
The two half-head partials per batch are summed on the host during gather.

Layout strategy (all matmuls contract over the partition dim, out = lhsT.T @ rhs):
  - x^T, y^T built once via PE transpose -> q^T,k^T = W.T @ x^T (K=C=64, N=512)
  - scores are computed TRANSPOSED: S^T[s,t] = k^T.T @ q^T  (K=D=64)
    so the AV matmul consumes exp(S^T) directly as its moving operand
    with K=s=128 (full PE) and no transposes in the hot loop.
  - v gets a ones column appended: AV lhsT = [v | 1] gives softmax row sums
    in partition row 64 of the o^T PSUM tile for free.
  - sums are batch-transposed (one small PE transpose per 128 t-columns)
    into per-partition layout, reciprocal'd once, and applied per head
    during the projection accumulation (scalar_tensor_tensor).
  - causality at 128-block granularity: s-tile j of t-chunk c only computes
    columns t >= 128j; the diagonal 128x128 triangle gets an additive -1e30
    mask before exp.
"""

import numpy as np

import concourse.bacc as bacc
import concourse.mybir as mybir
import concourse.tile as tile
from concourse.bass_utils import run_bass_kernel_spmd

P = 128
TCH = 512  # t-chunk (one PSUM bank of fp32)
NEG = -1e30
B, T, C, H, D = 4, 2048, 64, 6, 64
HPC = 3  # heads per core
NCORES = 8
F32 = mybir.dt.float32


def build_program(T=T, C=C, D=D, HPC=HPC, mm_dt=mybir.dt.float32r):
    assert T % TCH == 0
    NCH = T // TCH        # t-chunks of 512
    NB = T // P           # 128-blocks
    BPC = TCH // P        # 128-blocks per chunk (4)
    NSIG = NB             # output row-slices of 128
    nc = bacc.Bacc()

    x_d = nc.dram_tensor("x_loc", [T, C], F32, kind="ExternalInput")
    y_d = nc.dram_tensor("y_loc", [T, C], F32, kind="ExternalInput")
    wq_d = nc.dram_tensor("wq_loc", [HPC, C, D], F32, kind="ExternalInput")
    wk_d = nc.dram_tensor("wk_loc", [HPC, C, D], F32, kind="ExternalInput")
    wv_d = nc.dram_tensor("wv_loc", [HPC, C, D], F32, kind="ExternalInput")
    wp_d = nc.dram_tensor("wp_loc", [HPC, D, C], F32, kind="ExternalInput")
    bias_d = nc.dram_tensor("bias_loc", [1, C], F32, kind="ExternalInput")
    tri_d = nc.dram_tensor("tri_loc", [P, P], F32, kind="ExternalInput")
    ident_d = nc.dram_tensor("ident_loc", [P, P], F32, kind="ExternalInput")
    out_d = nc.dram_tensor("out_loc", [T, C], F32, kind="ExternalOutput")

    scale = float(D) ** -0.5
    NSR = HPC * NCH  # rows in the sums pack tile

    with tile.TileContext(nc) as tc:
        with (
            tc.tile_pool(name="const", bufs=1) as const,
            tc.tile_pool(name="big", bufs=1) as big,
            tc.tile_pool(name="work", bufs=3) as work,
            tc.tile_pool(name="ps_s", bufs=2, space="PSUM") as ps_s,
            tc.tile_pool(name="ps_o", bufs=2, space="PSUM") as ps_o,
            tc.tile_pool(name="ps_m", bufs=2, space="PSUM") as ps_m,
        ):
            # ---- constants -------------------------------------------------
            ident = const.tile([P, P], F32)
            nc.sync.dma_start(out=ident[:, :], in_=ident_d[:, :])
            tri = const.tile([P, P], F32)
            nc.sync.dma_start(out=tri[:, :], in_=tri_d[:, :])
            bias_row = const.tile([1, C], F32)
            nc.sync.dma_start(out=bias_row[:, :], in_=bias_d[:, :])
            bias = const.tile([P, C], F32)
            nc.gpsimd.partition_broadcast(bias[:, :], bias_row[:, :])
            wq_f = const.tile([C, HPC, D], F32)
            nc.sync.dma_start(out=wq_f[:], in_=wq_d[:].rearrange("h c d -> c h d"))
            wk_f = const.tile([C, HPC, D], F32)
            nc.sync.dma_start(out=wk_f[:], in_=wk_d[:].rearrange("h c d -> c h d"))
            wv_f = const.tile([C, HPC, D], F32)
            nc.sync.dma_start(out=wv_f[:], in_=wv_d[:].rearrange("h c d -> c h d"))
            wp_f = const.tile([D, HPC, C], F32)
            nc.sync.dma_start(out=wp_f[:], in_=wp_d[:].rearrange("h d c -> d h c"))
            wq_sb = const.tile([C, HPC, D], mm_dt)
            nc.vector.tensor_copy(wq_sb[:], wq_f[:])
            wk_sb = const.tile([C, HPC, D], mm_dt)
            nc.vector.tensor_copy(wk_sb[:], wk_f[:])
            wv_sb = const.tile([C, HPC, D], mm_dt)
            nc.vector.tensor_copy(wv_sb[:], wv_f[:])
            wp_sb = const.tile([D, HPC, C], mm_dt)
            nc.vector.tensor_copy(wp_sb[:], wp_f[:])

            x_sb = big.tile([P, NB, C], F32)
            nc.sync.dma_start(out=x_sb[:], in_=x_d[:].rearrange("(n p) c -> p n c", p=P))
            y_sb = big.tile([P, NB, C], F32)
            nc.sync.dma_start(out=y_sb[:], in_=y_d[:].rearrange("(n p) c -> p n c", p=P))

            # ---- x^T, y^T via PE transpose --------------------------------
            xT = big.tile([C, T], mm_dt)
            yT = big.tile([C, T], mm_dt)
            for src, dst in ((x_sb, xT), (y_sb, yT)):
                for j in range(NB):
                    tp = ps_m.tile([C, P], F32, tag="mps")
                    nc.tensor.transpose(tp[:, :], src[:, j, :], ident[:, :])
                    nc.vector.tensor_copy(dst[:, j * P : (j + 1) * P], tp[:, :])

            # ---- q^T, k^T, v ----------------------------------------------
            qT = big.tile([C, HPC, T], mm_dt)
            kT = big.tile([C, HPC, T], mm_dt)
            v_sb = big.tile([P, HPC, NB, D + 1], mm_dt)
            ones_col = const.tile([P, 1], F32)
            nc.vector.memset(ones_col[:, :], 1.0)
            nc.vector.tensor_copy(
                v_sb[:, :, :, D : D + 1],
                ones_col[:, :].to_broadcast((P, HPC, NB, 1)),
            )
            for h in range(HPC):
                for n4 in range(T // TCH):
                    sl = slice(n4 * TCH, (n4 + 1) * TCH)
                    qp = ps_m.tile([C, TCH], F32, tag="mps")
                    nc.tensor.matmul(
                        qp[:, :], lhsT=wq_sb[:, h, :],
                        rhs=xT[:, sl], start=True, stop=True,
                    )
                    nc.vector.tensor_copy(qT[:, h, sl], qp[:, :])
                    kp = ps_m.tile([C, TCH], F32, tag="mps")
                    nc.tensor.matmul(
                        kp[:, :], lhsT=wk_sb[:, h, :],
                        rhs=yT[:, sl], start=True, stop=True,
                    )
                    nc.vector.tensor_copy(kT[:, h, sl], kp[:, :])
                for g in range(NB // 8):
                    vp = ps_m.tile([P, 8 * D], F32, tag="mps")
                    for jj in range(8):
                        j = g * 8 + jj
                        nc.tensor.matmul(
                            vp[:, jj * D : (jj + 1) * D],
                            lhsT=yT[:, j * P : (j + 1) * P],
                            rhs=wv_sb[:, h, :], start=True, stop=True,
                        )
                    nc.vector.tensor_copy(
                        v_sb[:, h, g * 8 : (g + 1) * 8, 0:D],
                        vp[:, :].rearrange("p (a b) -> p a b", b=D),
                    )

            # ---- attention -------------------------------------------------
            # softmax sums (row 64 of the o^T psum tile) are collected per
            # head into rows 32*c of spack_h (DVE writes must start at a
            # partition offset in {0,32,64,96}).
            oT_sb = big.tile([C, HPC, T], mm_dt)
            SPP = 32 * (NCH - 1) + 1
            spacks = [big.tile([SPP, TCH], F32, name=f"spack{h}") for h in range(HPC)]
            for h in range(HPC):
                for c in range(NCH):
                    ns = BPC * c + BPC
                    ops = ps_o.tile([D + 1, TCH], F32, tag="oT")
                    for j in range(ns):
                        t0 = max(0, P * j - TCH * c)
                        sps = ps_s.tile([P, TCH], F32, tag="S")
                        nc.tensor.matmul(
                            sps[:, t0:TCH],
                            lhsT=kT[:, h, j * P : (j + 1) * P],
                            rhs=qT[:, h, c * TCH + t0 : (c + 1) * TCH],
                            start=True, stop=True,
                        )
                        if j >= BPC * c:  # diagonal tile: mask the triangle
                            nc.vector.tensor_add(
                                sps[:, t0 : t0 + P], sps[:, t0 : t0 + P], tri[:, :]
                            )
                        es = work.tile([P, TCH], mm_dt, tag="expS")
                        nc.scalar.activation(
                            es[:, t0:TCH], sps[:, t0:TCH],
                            mybir.ActivationFunctionType.Exp, scale=scale,
                        )
                        nc.tensor.matmul(
                            ops[:, t0:TCH],
                            lhsT=v_sb[:, h, j, :],
                            rhs=es[:, t0:TCH],
                            start=(j == 0), stop=(j == ns - 1),
                            skip_group_check=True,
                        )
                    nc.vector.tensor_copy(
                        oT_sb[:, h, c * TCH : (c + 1) * TCH], ops[0:C, :]
                    )
                    nc.vector.tensor_copy(
                        spacks[h][32 * c : 32 * c + 1, :], ops[D : D + 1, :]
                    )

            # ---- reciprocal of softmax sums -------------------------------
            # transpose each head's [97, 512] sums pack (valid rows 32c) into
            # [128, 97] psum blocks; reciprocal only the valid strided columns.
            recip = big.tile([P, HPC, BPC, NCH], F32)
            for h in range(HPC):
                rp = ps_s.tile([P, TCH], F32, tag="S", name=f"rps{h}")
                for sc in range(BPC):
                    nc.tensor.transpose(
                        rp[:, sc * P : sc * P + SPP],
                        spacks[h][0:SPP, sc * P : (sc + 1) * P],
                        ident[0:SPP, 0:SPP],
                    )
                rp_valid = rp[:, :].rearrange("p (a b) -> p a b", a=BPC)[:, :, 0 : 32 * NCH : 32]
                nc.vector.reciprocal(recip[:, h, :, :], rp_valid)

            # ---- projection + normalization + bias ------------------------
            out_full = big.tile([P, NSIG, C], F32)
            for sg in range(NSIG):
                c, sc = sg // BPC, sg % BPC
                pp = ps_m.tile([P, HPC * C], F32, tag="mps")
                for h in range(HPC):
                    nc.tensor.matmul(
                        pp[:, h * C : (h + 1) * C],
                        lhsT=oT_sb[:, h, sg * P : (sg + 1) * P],
                        rhs=wp_sb[:, h, :], start=True, stop=True,
                    )
                acc = bias[:, :]
                for h in range(HPC):
                    dst = (
                        out_full[:, sg, :]
                        if h == HPC - 1
                        else work.tile([P, C], F32, tag="acc", name="acc")[:, :]
                    )
                    nc.vector.scalar_tensor_tensor(
                        out=dst,
                        in0=pp[:, h * C : (h + 1) * C],
                        scalar=recip[:, h, sc, c : c + 1],
                        in1=acc,
                        op0=mybir.AluOpType.mult,
                        op1=mybir.AluOpType.add,
                    )
                    acc = dst
            nc.sync.dma_start(
                out=out_d[:].rearrange("(n p) c -> p n c", p=P), in_=out_full[:]
            )

    nc.compile()
    return nc


_prog = None


def _get_program():
    global _prog
    if _prog is None:
        _prog = build_program()
    return _prog


def make_host_consts():
    i = np.arange(P)
    # tri[s_rel, t_rel]: valid (0) when t_rel >= s_rel else -1e30
    tri = np.where(i[None, :] >= i[:, None], 0.0, NEG).astype(np.float32)
    ident = np.eye(P, dtype=np.float32)
    return tri, ident


def make_in_maps(inputs=None, **kw):
    if inputs is None:
        inputs = kw
    x = np.ascontiguousarray(np.asarray(inputs["x"], np.float32))
    y = np.ascontiguousarray(np.asarray(inputs["y"], np.float32))
    Wq = np.ascontiguousarray(np.asarray(inputs["Wq"], np.float32))
    Wk = np.ascontiguousarray(np.asarray(inputs["Wk"], np.float32))
    Wv = np.ascontiguousarray(np.asarray(inputs["Wv"], np.float32))
    Wp = np.ascontiguousarray(np.asarray(inputs["W_proj"], np.float32)).reshape(
        H, D, C
    )
    b_proj = np.asarray(inputs["b_proj"], np.float32)
    tri, ident = make_host_consts()
    zeros_c = np.zeros_like(b_proj)

    in_maps = []
    for core in range(NCORES):
        b, half = core // 2, core % 2
        h0 = HPC * half
        in_maps.append(
            {
                "x_loc": x[b],
                "y_loc": y[b],
                "wq_loc": Wq[h0 : h0 + HPC],
                "wk_loc": Wk[h0 : h0 + HPC],
                "wv_loc": Wv[h0 : h0 + HPC],
                "wp_loc": Wp[h0 : h0 + HPC],
                "bias_loc": (b_proj if half == 0 else zeros_c)[None, :],
                "tri_loc": tri,
                "ident_loc": ident,
            }
        )
    return in_maps


def assemble(results):
    out = np.stack(
        [results[2 * b]["out_loc"] + results[2 * b + 1]["out_loc"] for b in range(B)]
    )
    return out.astype(np.float32)


def kernel(x, y, Wq, Wk, Wv, W_proj, b_proj):
    in_maps = make_in_maps(
        x=x, y=y, Wq=Wq, Wk=Wk, Wv=Wv, W_proj=W_proj, b_proj=b_proj
    )
    nc = _get_program()
    res = run_bass_kernel_spmd(nc, in_maps, list(range(NCORES))).results
    return assemble(res)


# revision 13
# speedup vs baseline: 1.0295x; 1.0295x over previous
"""Trainium2 Bass kernel: MultiHeadCrossAttention (B=4, T=2048, C=64, H=6, D=64).

Sharding: 8 cores = 4 batches x 2 head-groups of 3. Every core runs an
identical (SPMD) program: causal flash-attention for 3 heads over the full
T=2048 sequence of one batch, followed by its partial output projection.
The two half-head partials per batch are summed on the host during gather.

Layout strategy (all matmuls contract over the partition dim, out = lhsT.T @ rhs):
  - x^T, y^T built once via PE transpose -> q^T,k^T = W.T @ x^T (K=C=64, N=512)
  - scores are computed TRANSPOSED: S^T[s,t] = k^T.T @ q^T  (K=D=64)
    so the AV matmul consumes exp(S^T) directly as its moving operand
    with K=s=128 (full PE) and no transposes in the hot loop.
  - v gets a ones column appended: AV lhsT = [v | 1] gives softmax row sums
    in partition row 64 of the o^T PSUM tile for free.
  - sums are batch-transposed (one small PE transpose per 128 t-columns)
    into per-partition layout, reciprocal'd once, and applied per head
    during the projection accumulation (scalar_tensor_tensor).
  - causality at 128-block granularity: s-tile j of t-chunk c only computes
    columns t >= 128j; the diagonal 128x128 triangle gets an additive -1e30
    mask before exp.
"""

import numpy as np

import concourse.bacc as bacc
import concourse.mybir as mybir
import concourse.tile as tile
from concourse.bass_utils import run_bass_kernel_spmd

P = 128
TCH = 512  # t-chunk (one PSUM bank of fp32)
NEG = -1e30
B, T, C, H, D = 4, 2048, 64, 6, 64
HPC = 3  # heads per core
NCORES = 8
F32 = mybir.dt.float32


def build_program(T=T, C=C, D=D, HPC=HPC, mm_dt=mybir.dt.float32r, av_dt=mybir.dt.float16):
    assert T % TCH == 0
    NCH = T // TCH        # t-chunks of 512
    NB = T // P           # 128-blocks
    BPC = TCH // P        # 128-blocks per chunk (4)
    NSIG = NB             # output row-slices of 128
    nc = bacc.Bacc()

    x_d = nc.dram_tensor("x_loc", [T, C], F32, kind="ExternalInput")
    y_d = nc.dram_tensor("y_loc", [T, C], F32, kind="ExternalInput")
    wq_d = nc.dram_tensor("wq_loc", [HPC, C, D], F32, kind="ExternalInput")
    wk_d = nc.dram_tensor("wk_loc", [HPC, C, D], F32, kind="ExternalInput")
    wv_d = nc.dram_tensor("wv_loc", [HPC, C, D], F32, kind="ExternalInput")
    wp_d = nc.dram_tensor("wp_loc", [HPC, D, C], F32, kind="ExternalInput")
    bias_d = nc.dram_tensor("bias_loc", [1, C], F32, kind="ExternalInput")
    tri_d = nc.dram_tensor("tri_loc", [P, P], F32, kind="ExternalInput")
    ident_d = nc.dram_tensor("ident_loc", [P, P], F32, kind="ExternalInput")
    out_d = nc.dram_tensor("out_loc", [T, C], F32, kind="ExternalOutput")

    scale = float(D) ** -0.5
    NSR = HPC * NCH  # rows in the sums pack tile

    with tile.TileContext(nc) as tc:
        with (
            tc.tile_pool(name="const", bufs=1) as const,
            tc.tile_pool(name="big", bufs=1) as big,
            tc.tile_pool(name="work", bufs=3) as work,
            tc.tile_pool(name="ps_s", bufs=2, space="PSUM") as ps_s,
            tc.tile_pool(name="ps_o", bufs=2, space="PSUM") as ps_o,
            tc.tile_pool(name="ps_m", bufs=2, space="PSUM") as ps_m,
        ):
            # ---- constants -------------------------------------------------
            ident = const.tile([P, P], F32)
            nc.sync.dma_start(out=ident[:, :], in_=ident_d[:, :])
            tri = const.tile([P, P], F32)
            nc.sync.dma_start(out=tri[:, :], in_=tri_d[:, :])
            bias_row = const.tile([1, C], F32)
            nc.sync.dma_start(out=bias_row[:, :], in_=bias_d[:, :])
            bias = const.tile([P, C], F32)
            nc.gpsimd.partition_broadcast(bias[:, :], bias_row[:, :])
            wq_f = const.tile([C, HPC, D], F32)
            nc.sync.dma_start(out=wq_f[:], in_=wq_d[:].rearrange("h c d -> c h d"))
            wk_f = const.tile([C, HPC, D], F32)
            nc.sync.dma_start(out=wk_f[:], in_=wk_d[:].rearrange("h c d -> c h d"))
            wv_f = const.tile([C, HPC, D], F32)
            nc.sync.dma_start(out=wv_f[:], in_=wv_d[:].rearrange("h c d -> c h d"))
            wp_f = const.tile([D, HPC, C], F32)
            nc.sync.dma_start(out=wp_f[:], in_=wp_d[:].rearrange("h d c -> d h c"))
            wq_sb = const.tile([C, HPC, D], mm_dt)
            nc.vector.tensor_copy(wq_sb[:], wq_f[:])
            wk_sb = const.tile([C, HPC, D], mm_dt)
            nc.vector.tensor_copy(wk_sb[:], wk_f[:])
            wv_sb = const.tile([C, HPC, D], mm_dt)
            nc.vector.tensor_copy(wv_sb[:], wv_f[:])
            wp_sb = const.tile([D, HPC, C], mm_dt)
            nc.vector.tensor_copy(wp_sb[:], wp_f[:])

            x_sb = big.tile([P, NB, C], F32)
            nc.sync.dma_start(out=x_sb[:], in_=x_d[:].rearrange("(n p) c -> p n c", p=P))
            y_sb = big.tile([P, NB, C], F32)
            nc.sync.dma_start(out=y_sb[:], in_=y_d[:].rearrange("(n p) c -> p n c", p=P))

            # ---- x^T, y^T via PE transpose --------------------------------
            xT = big.tile([C, T], mm_dt)
            yT = big.tile([C, T], mm_dt)
            for src, dst in ((x_sb, xT), (y_sb, yT)):
                for j in range(NB):
                    tp = ps_m.tile([C, P], F32, tag="mps")
                    nc.tensor.transpose(tp[:, :], src[:, j, :], ident[:, :])
                    nc.vector.tensor_copy(dst[:, j * P : (j + 1) * P], tp[:, :])

            # ---- q^T, k^T, v ----------------------------------------------
            qT = big.tile([C, HPC, T], mm_dt)
            kT = big.tile([C, HPC, T], mm_dt)
            v_sb = big.tile([P, HPC, NB, D + 1], av_dt)
            ones_col = const.tile([P, 1], F32)
            nc.vector.memset(ones_col[:, :], 1.0)
            nc.vector.tensor_copy(
                v_sb[:, :, :, D : D + 1],
                ones_col[:, :].to_broadcast((P, HPC, NB, 1)),
            )
            for h in range(HPC):
                for n4 in range(T // TCH):
                    sl = slice(n4 * TCH, (n4 + 1) * TCH)
                    qp = ps_m.tile([C, TCH], F32, tag="mps")
                    nc.tensor.matmul(
                        qp[:, :], lhsT=wq_sb[:, h, :],
                        rhs=xT[:, sl], start=True, stop=True,
                    )
                    nc.vector.tensor_copy(qT[:, h, sl], qp[:, :])
                    kp = ps_m.tile([C, TCH], F32, tag="mps")
                    nc.tensor.matmul(
                        kp[:, :], lhsT=wk_sb[:, h, :],
                        rhs=yT[:, sl], start=True, stop=True,
                    )
                    nc.vector.tensor_copy(kT[:, h, sl], kp[:, :])
                for g in range(NB // 8):
                    vp = ps_m.tile([P, 8 * D], F32, tag="mps")
                    for jj in range(8):
                        j = g * 8 + jj
                        nc.tensor.matmul(
                            vp[:, jj * D : (jj + 1) * D],
                            lhsT=yT[:, j * P : (j + 1) * P],
                            rhs=wv_sb[:, h, :], start=True, stop=True,
                        )
                    nc.vector.tensor_copy(
                        v_sb[:, h, g * 8 : (g + 1) * 8, 0:D],
                        vp[:, :].rearrange("p (a b) -> p a b", b=D),
                    )

            # ---- attention -------------------------------------------------
            # softmax sums (row 64 of the o^T psum tile) are collected per
            # head into rows 32*c of spack_h (DVE writes must start at a
            # partition offset in {0,32,64,96}).
            oT_sb = big.tile([C, HPC, T], mm_dt)
            SPP = 32 * (NCH - 1) + 1
            spacks = [big.tile([SPP, TCH], F32, name=f"spack{h}") for h in range(HPC)]
            for h in range(HPC):
                for c in range(NCH):
                    ns = BPC * c + BPC
                    ops = ps_o.tile([D + 1, TCH], F32, tag="oT")
                    for j in range(ns):
                        t0 = max(0, P * j - TCH * c)
                        sps = ps_s.tile([P, TCH], F32, tag="S")
                        nc.tensor.matmul(
                            sps[:, t0:TCH],
                            lhsT=kT[:, h, j * P : (j + 1) * P],
                            rhs=qT[:, h, c * TCH + t0 : (c + 1) * TCH],
                            start=True, stop=True,
                        )
                        if j >= BPC * c:  # diagonal tile: mask the triangle
                            nc.vector.tensor_add(
                                sps[:, t0 : t0 + P], sps[:, t0 : t0 + P], tri[:, :]
                            )
                        es = work.tile([P, TCH], av_dt, tag="expS")
                        nc.scalar.activation(
                            es[:, t0:TCH], sps[:, t0:TCH],
                            mybir.ActivationFunctionType.Exp, scale=scale,
                        )
                        nc.tensor.matmul(
                            ops[:, t0:TCH],
                            lhsT=v_sb[:, h, j, :],
                            rhs=es[:, t0:TCH],
                            start=(j == 0), stop=(j == ns - 1),
                            skip_group_check=True,
                        )
                    nc.vector.tensor_copy(
                        oT_sb[:, h, c * TCH : (c + 1) * TCH], ops[0:C, :]
                    )
                    nc.vector.tensor_copy(
                        spacks[h][32 * c : 32 * c + 1, :], ops[D : D + 1, :]
                    )

            # ---- reciprocal of softmax sums -------------------------------
            # transpose each head's [97, 512] sums pack (valid rows 32c) into
            # [128, 97] psum blocks; reciprocal only the valid strided columns.
            recip = big.tile([P, HPC, BPC, NCH], F32)
            for h in range(HPC):
                rp = ps_s.tile([P, TCH], F32, tag="S", name=f"rps{h}")
                for sc in range(BPC):
                    nc.tensor.transpose(
                        rp[:, sc * P : sc * P + SPP],
                        spacks[h][0:SPP, sc * P : (sc + 1) * P],
                        ident[0:SPP, 0:SPP],
                    )
                rp_valid = rp[:, :].rearrange("p (a b) -> p a b", a=BPC)[:, :, 0 : 32 * NCH : 32]
                nc.vector.reciprocal(recip[:, h, :, :], rp_valid)

            # ---- projection + normalization + bias ------------------------
            out_full = big.tile([P, NSIG, C], F32)
            for sg in range(NSIG):
                c, sc = sg // BPC, sg % BPC
                pp = ps_m.tile([P, HPC * C], F32, tag="mps")
                for h in range(HPC):
                    nc.tensor.matmul(
                        pp[:, h * C : (h + 1) * C],
                        lhsT=oT_sb[:, h, sg * P : (sg + 1) * P],
                        rhs=wp_sb[:, h, :], start=True, stop=True,
                    )
                acc = bias[:, :]
                for h in range(HPC):
                    dst = (
                        out_full[:, sg, :]
                        if h == HPC - 1
                        else work.tile([P, C], F32, tag="acc", name="acc")[:, :]
                    )
                    nc.vector.scalar_tensor_tensor(
                        out=dst,
                        in0=pp[:, h * C : (h + 1) * C],
                        scalar=recip[:, h, sc, c : c + 1],
                        in1=acc,
                        op0=mybir.AluOpType.mult,
                        op1=mybir.AluOpType.add,
                    )
                    acc = dst
            nc.sync.dma_start(
                out=out_d[:].rearrange("(n p) c -> p n c", p=P), in_=out_full[:]
            )

    nc.compile()
    return nc


_prog = None


def _get_program():
    global _prog
    if _prog is None:
        _prog = build_program()
    return _prog


def make_host_consts():
    i = np.arange(P)
    # tri[s_rel, t_rel]: valid (0) when t_rel >= s_rel else -1e30
    tri = np.where(i[None, :] >= i[:, None], 0.0, NEG).astype(np.float32)
    ident = np.eye(P, dtype=np.float32)
    return tri, ident


def make_in_maps(inputs=None, **kw):
    if inputs is None:
        inputs = kw
    x = np.ascontiguousarray(np.asarray(inputs["x"], np.float32))
    y = np.ascontiguousarray(np.asarray(inputs["y"], np.float32))
    Wq = np.ascontiguousarray(np.asarray(inputs["Wq"], np.float32))
    Wk = np.ascontiguousarray(np.asarray(inputs["Wk"], np.float32))
    Wv = np.ascontiguousarray(np.asarray(inputs["Wv"], np.float32))
    Wp = np.ascontiguousarray(np.asarray(inputs["W_proj"], np.float32)).reshape(
        H, D, C
    )
    b_proj = np.asarray(inputs["b_proj"], np.float32)
    tri, ident = make_host_consts()
    zeros_c = np.zeros_like(b_proj)

    in_maps = []
    for core in range(NCORES):
        b, half = core // 2, core % 2
        h0 = HPC * half
        in_maps.append(
            {
                "x_loc": x[b],
                "y_loc": y[b],
                "wq_loc": Wq[h0 : h0 + HPC],
                "wk_loc": Wk[h0 : h0 + HPC],
                "wv_loc": Wv[h0 : h0 + HPC],
                "wp_loc": Wp[h0 : h0 + HPC],
                "bias_loc": (b_proj if half == 0 else zeros_c)[None, :],
                "tri_loc": tri,
                "ident_loc": ident,
            }
        )
    return in_maps


def assemble(results):
    out = np.stack(
        [results[2 * b]["out_loc"] + results[2 * b + 1]["out_loc"] for b in range(B)]
    )
    return out.astype(np.float32)


def kernel(x, y, Wq, Wk, Wv, W_proj, b_proj):
    in_maps = make_in_maps(
        x=x, y=y, Wq=Wq, Wk=Wk, Wv=Wv, W_proj=W_proj, b_proj=b_proj
    )
    nc = _get_program()
    res = run_bass_kernel_spmd(nc, in_maps, list(range(NCORES))).results
    return assemble(res)


# revision 14
# speedup vs baseline: 1.0301x; 1.0007x over previous
"""Trainium2 Bass kernel: MultiHeadCrossAttention (B=4, T=2048, C=64, H=6, D=64).

Sharding: 8 cores = 4 batches x 2 head-groups of 3. Every core runs an
identical (SPMD) program: causal flash-attention for 3 heads over the full
T=2048 sequence of one batch, followed by its partial output projection.
The two half-head partials per batch are summed on the host during gather.

Layout strategy (all matmuls contract over the partition dim, out = lhsT.T @ rhs):
  - x^T, y^T built once via PE transpose -> q^T,k^T = W.T @ x^T (K=C=64, N=512)
  - scores are computed TRANSPOSED: S^T[s,t] = k^T.T @ q^T  (K=D=64)
    so the AV matmul consumes exp(S^T) directly as its moving operand
    with K=s=128 (full PE) and no transposes in the hot loop.
  - v gets a ones column appended: AV lhsT = [v | 1] gives softmax row sums
    in partition row 64 of the o^T PSUM tile for free.
  - sums are batch-transposed (one small PE transpose per 128 t-columns)
    into per-partition layout, reciprocal'd once, and applied per head
    during the projection accumulation (scalar_tensor_tensor).
  - causality at 128-block granularity: s-tile j of t-chunk c only computes
    columns t >= 128j; the diagonal 128x128 triangle gets an additive -1e30
    mask before exp.
"""

import numpy as np

import concourse.bacc as bacc
import concourse.mybir as mybir
import concourse.tile as tile
from concourse.bass_utils import run_bass_kernel_spmd

P = 128
TCH = 512  # t-chunk (one PSUM bank of fp32)
NEG = -1e30
B, T, C, H, D = 4, 2048, 64, 6, 64
HPC = 3  # heads per core
NCORES = 8
F32 = mybir.dt.float32


def build_program(T=T, C=C, D=D, HPC=HPC, mm_dt=mybir.dt.float32r, av_dt=mybir.dt.float32r):
    assert T % TCH == 0
    NCH = T // TCH        # t-chunks of 512
    NB = T // P           # 128-blocks
    BPC = TCH // P        # 128-blocks per chunk (4)
    NSIG = NB             # output row-slices of 128
    nc = bacc.Bacc()

    x_d = nc.dram_tensor("x_loc", [T, C], F32, kind="ExternalInput")
    y_d = nc.dram_tensor("y_loc", [T, C], F32, kind="ExternalInput")
    wq_d = nc.dram_tensor("wq_loc", [HPC, C, D], F32, kind="ExternalInput")
    wk_d = nc.dram_tensor("wk_loc", [HPC, C, D], F32, kind="ExternalInput")
    wv_d = nc.dram_tensor("wv_loc", [HPC, C, D], F32, kind="ExternalInput")
    wp_d = nc.dram_tensor("wp_loc", [HPC, D, C], F32, kind="ExternalInput")
    bias_d = nc.dram_tensor("bias_loc", [1, C], F32, kind="ExternalInput")
    tri_d = nc.dram_tensor("tri_loc", [P, P], F32, kind="ExternalInput")
    ident_d = nc.dram_tensor("ident_loc", [P, P], F32, kind="ExternalInput")
    out_d = nc.dram_tensor("out_loc", [T, C], F32, kind="ExternalOutput")

    scale = float(D) ** -0.5
    NSR = HPC * NCH  # rows in the sums pack tile

    with tile.TileContext(nc) as tc:
        with (
            tc.tile_pool(name="const", bufs=1) as const,
            tc.tile_pool(name="big", bufs=1) as big,
            tc.tile_pool(name="work", bufs=3) as work,
            tc.tile_pool(name="ps_s", bufs=4, space="PSUM") as ps_s,
            tc.tile_pool(name="ps_o", bufs=2, space="PSUM") as ps_o,
            tc.tile_pool(name="ps_m", bufs=2, space="PSUM") as ps_m,
        ):
            # ---- constants -------------------------------------------------
            ident = const.tile([P, P], F32)
            nc.sync.dma_start(out=ident[:, :], in_=ident_d[:, :])
            tri = const.tile([P, P], F32)
            nc.sync.dma_start(out=tri[:, :], in_=tri_d[:, :])
            bias_row = const.tile([1, C], F32)
            nc.sync.dma_start(out=bias_row[:, :], in_=bias_d[:, :])
            bias = const.tile([P, C], F32)
            nc.gpsimd.partition_broadcast(bias[:, :], bias_row[:, :])
            wq_f = const.tile([C, HPC, D], F32)
            nc.sync.dma_start(out=wq_f[:], in_=wq_d[:].rearrange("h c d -> c h d"))
            wk_f = const.tile([C, HPC, D], F32)
            nc.sync.dma_start(out=wk_f[:], in_=wk_d[:].rearrange("h c d -> c h d"))
            wv_f = const.tile([C, HPC, D], F32)
            nc.sync.dma_start(out=wv_f[:], in_=wv_d[:].rearrange("h c d -> c h d"))
            wp_f = const.tile([D, HPC, C], F32)
            nc.sync.dma_start(out=wp_f[:], in_=wp_d[:].rearrange("h d c -> d h c"))
            wq_sb = const.tile([C, HPC, D], mm_dt)
            nc.vector.tensor_copy(wq_sb[:], wq_f[:])
            wk_sb = const.tile([C, HPC, D], mm_dt)
            nc.vector.tensor_copy(wk_sb[:], wk_f[:])
            wv_sb = const.tile([C, HPC, D], mm_dt)
            nc.vector.tensor_copy(wv_sb[:], wv_f[:])
            wp_sb = const.tile([D, HPC, C], mm_dt)
            nc.vector.tensor_copy(wp_sb[:], wp_f[:])

            x_sb = big.tile([P, NB, C], F32)
            nc.sync.dma_start(out=x_sb[:], in_=x_d[:].rearrange("(n p) c -> p n c", p=P))
            y_sb = big.tile([P, NB, C], F32)
            nc.sync.dma_start(out=y_sb[:], in_=y_d[:].rearrange("(n p) c -> p n c", p=P))

            # ---- x^T, y^T via PE transpose --------------------------------
            xT = big.tile([C, T], mm_dt)
            yT = big.tile([C, T], mm_dt)
            for src, dst in ((x_sb, xT), (y_sb, yT)):
                for j in range(NB):
                    tp = ps_m.tile([C, P], F32, tag="mps")
                    nc.tensor.transpose(tp[:, :], src[:, j, :], ident[:, :])
                    nc.vector.tensor_copy(dst[:, j * P : (j + 1) * P], tp[:, :])

            # ---- q^T, k^T, v ----------------------------------------------
            # stacked pair layouts: partitions 0-63 hold q/k for even
            # s-blocks (row group 0), 64-127 for odd s-blocks (row group 64),
            # enabling concurrent S matmuls on both PE row halves.
            qT2 = big.tile([2 * C, HPC, T], mm_dt)
            kT2 = big.tile([2 * C, HPC, (NB // 2) * P], mm_dt)
            v_sb = big.tile([P, HPC, NB, D + 1], av_dt)
            ones_col = const.tile([P, 1], F32)
            nc.vector.memset(ones_col[:, :], 1.0)
            nc.vector.tensor_copy(
                v_sb[:, :, :, D : D + 1],
                ones_col[:, :].to_broadcast((P, HPC, NB, 1)),
            )
            for h in range(HPC):
                for n4 in range(T // TCH):
                    sl = slice(n4 * TCH, (n4 + 1) * TCH)
                    qp = ps_m.tile([C, TCH], F32, tag="mps")
                    nc.tensor.matmul(
                        qp[:, :], lhsT=wq_sb[:, h, :],
                        rhs=xT[:, sl], start=True, stop=True,
                    )
                    nc.vector.tensor_copy(qT2[0:C, h, sl], qp[:, :])
                    nc.vector.tensor_copy(qT2[C : 2 * C, h, sl], qp[:, :])
                    kp = ps_m.tile([C, TCH], F32, tag="mps")
                    nc.tensor.matmul(
                        kp[:, :], lhsT=wk_sb[:, h, :],
                        rhs=yT[:, sl], start=True, stop=True,
                    )
                    kpv = kp[:, :].rearrange("p (a b) -> p a b", b=P)
                    kdst = kT2[:, h, 2 * n4 * P : (2 * n4 + 2) * P].rearrange(
                        "p (a b) -> p a b", b=P
                    )
                    nc.vector.tensor_copy(kdst[0:C], kpv[:, 0:4:2, :])
                    nc.vector.tensor_copy(kdst[C : 2 * C], kpv[:, 1:4:2, :])
                for g in range(NB // 8):
                    vp = ps_m.tile([P, 8 * D], F32, tag="mps")
                    for jj in range(8):
                        j = g * 8 + jj
                        nc.tensor.matmul(
                            vp[:, jj * D : (jj + 1) * D],
                            lhsT=yT[:, j * P : (j + 1) * P],
                            rhs=wv_sb[:, h, :], start=True, stop=True,
                        )
                    nc.vector.tensor_copy(
                        v_sb[:, h, g * 8 : (g + 1) * 8, 0:D],
                        vp[:, :].rearrange("p (a b) -> p a b", b=D),
                    )

            # ---- attention -------------------------------------------------
            # softmax sums (row 64 of the o^T psum tile) are collected per
            # head into rows 32*c of spack_h (DVE writes must start at a
            # partition offset in {0,32,64,96}).
            oT_sb = big.tile([C, HPC, T], mm_dt)
            SPP = 32 * (NCH - 1) + 1
            spacks = [big.tile([SPP, TCH], F32, name=f"spack{h}") for h in range(HPC)]
            for h in range(HPC):
                for c in range(NCH):
                    ns = BPC * c + BPC
                    ops = ps_o.tile([D + 1, TCH], F32, tag="oT")
                    for pr in range(ns // 2):
                        jA, jB = 2 * pr, 2 * pr + 1
                        sub = []
                        for half, j in ((0, jA), (1, jB)):
                            t0 = max(0, P * j - TCH * c)
                            sps = ps_s.tile([P, TCH], F32, tag="S", name=f"sps{half}")
                            nc.tensor.matmul(
                                sps[:, t0:TCH],
                                lhsT=kT2[half * C : (half + 1) * C, h,
                                         pr * P : (pr + 1) * P],
                                rhs=qT2[half * C : (half + 1) * C, h,
                                        c * TCH + t0 : (c + 1) * TCH],
                                start=True, stop=True,
                            )
                            sub.append((j, t0, sps))
                        for j, t0, sps in sub:
                            if j >= BPC * c:  # diagonal tile: mask the triangle
                                nc.vector.tensor_add(
                                    sps[:, t0 : t0 + P], sps[:, t0 : t0 + P], tri[:, :]
                                )
                            es = work.tile([P, TCH], av_dt, tag="expS")
                            nc.scalar.activation(
                                es[:, t0:TCH], sps[:, t0:TCH],
                                mybir.ActivationFunctionType.Exp, scale=scale,
                            )
                            nc.tensor.matmul(
                                ops[:, t0:TCH],
                                lhsT=v_sb[:, h, j, :],
                                rhs=es[:, t0:TCH],
                                start=(j == 0), stop=(j == ns - 1),
                                skip_group_check=True,
                            )
                    nc.vector.tensor_copy(
                        oT_sb[:, h, c * TCH : (c + 1) * TCH], ops[0:C, :]
                    )
                    nc.vector.tensor_copy(
                        spacks[h][32 * c : 32 * c + 1, :], ops[D : D + 1, :]
                    )

            # ---- reciprocal of softmax sums -------------------------------
            # transpose each head's [97, 512] sums pack (valid rows 32c) into
            # [128, 97] psum blocks; reciprocal only the valid strided columns.
            recip = big.tile([P, HPC, BPC, NCH], F32)
            for h in range(HPC):
                rp = ps_s.tile([P, TCH], F32, tag="S", name=f"rps{h}")
                for sc in range(BPC):
                    nc.tensor.transpose(
                        rp[:, sc * P : sc * P + SPP],
                        spacks[h][0:SPP, sc * P : (sc + 1) * P],
                        ident[0:SPP, 0:SPP],
                    )
                rp_valid = rp[:, :].rearrange("p (a b) -> p a b", a=BPC)[:, :, 0 : 32 * NCH : 32]
                nc.vector.reciprocal(recip[:, h, :, :], rp_valid)

            # ---- projection + normalization + bias ------------------------
            out_full = big.tile([P, NSIG, C], F32)
            for sg in range(NSIG):
                c, sc = sg // BPC, sg % BPC
                pp = ps_m.tile([P, HPC * C], F32, tag="mps")
                for h in range(HPC):
                    nc.tensor.matmul(
                        pp[:, h * C : (h + 1) * C],
                        lhsT=oT_sb[:, h, sg * P : (sg + 1) * P],
                        rhs=wp_sb[:, h, :], start=True, stop=True,
                    )
                acc = bias[:, :]
                for h in range(HPC):
                    dst = (
                        out_full[:, sg, :]
                        if h == HPC - 1
                        else work.tile([P, C], F32, tag="acc", name="acc")[:, :]
                    )
                    nc.vector.scalar_tensor_tensor(
                        out=dst,
                        in0=pp[:, h * C : (h + 1) * C],
                        scalar=recip[:, h, sc, c : c + 1],
                        in1=acc,
                        op0=mybir.AluOpType.mult,
                        op1=mybir.AluOpType.add,
                    )
                    acc = dst
            nc.sync.dma_start(
                out=out_d[:].rearrange("(n p) c -> p n c", p=P), in_=out_full[:]
            )

    nc.compile()
    return nc


_prog = None


def _get_program():
    global _prog
    if _prog is None:
        _prog = build_program()
    return _prog


def make_host_consts():
    i = np.arange(P)
    # tri[s_rel, t_rel]: valid (0) when t_rel >= s_rel else -1e30
    tri = np.where(i[None, :] >= i[:, None], 0.0, NEG).astype(np.float32)
    ident = np.eye(P, dtype=np.float32)
    return tri, ident


def make_in_maps(inputs=None, **kw):
    if inputs is None:
        inputs = kw
    x = np.ascontiguousarray(np.asarray(inputs["x"], np.float32))
    y = np.ascontiguousarray(np.asarray(inputs["y"], np.float32))
    Wq = np.ascontiguousarray(np.asarray(inputs["Wq"], np.float32))
    Wk = np.ascontiguousarray(np.asarray(inputs["Wk"], np.float32))
    Wv = np.ascontiguousarray(np.asarray(inputs["Wv"], np.float32))
    Wp = np.ascontiguousarray(np.asarray(inputs["W_proj"], np.float32)).reshape(
        H, D, C
    )
    b_proj = np.asarray(inputs["b_proj"], np.float32)
    tri, ident = make_host_consts()
    zeros_c = np.zeros_like(b_proj)

    in_maps = []
    for core in range(NCORES):
        b, half = core // 2, core % 2
        h0 = HPC * half
        in_maps.append(
            {
                "x_loc": x[b],
                "y_loc": y[b],
                "wq_loc": Wq[h0 : h0 + HPC],
                "wk_loc": Wk[h0 : h0 + HPC],
                "wv_loc": Wv[h0 : h0 + HPC],
                "wp_loc": Wp[h0 : h0 + HPC],
                "bias_loc": (b_proj if half == 0 else zeros_c)[None, :],
                "tri_loc": tri,
                "ident_loc": ident,
            }
        )
    return in_maps


def assemble(results):
    out = np.stack(
        [results[2 * b]["out_loc"] + results[2 * b + 1]["out_loc"] for b in range(B)]
    )
    return out.astype(np.float32)


def kernel(x, y, Wq, Wk, Wv, W_proj, b_proj):
    in_maps = make_in_maps(
        x=x, y=y, Wq=Wq, Wk=Wk, Wv=Wv, W_proj=W_proj, b_proj=b_proj
    )
    nc = _get_program()
    res = run_bass_kernel_spmd(nc, in_maps, list(range(NCORES))).results
    return assemble(res)


# revision 15
# speedup vs baseline: 1.0862x; 1.0545x over previous
"""Trainium2 Bass kernel: MultiHeadCrossAttention (B=4, T=2048, C=64, H=6, D=64).

Sharding: 8 cores = 4 batches x 2 head-groups of 3. Every core runs an
identical (SPMD) program: causal flash-attention for 3 heads over the full
T=2048 sequence of one batch, followed by its partial output projection.
The two half-head partials per batch are summed on the host during gather.

Layout strategy (all matmuls contract over the partition dim, out = lhsT.T @ rhs):
  - x^T, y^T built once via PE transpose -> q^T,k^T = W.T @ x^T (K=C=64, N=512)
  - scores are computed TRANSPOSED: S^T[s,t] = k^T.T @ q^T  (K=D=64)
    so the AV matmul consumes exp(S^T) directly as its moving operand
    with K=s=128 (full PE) and no transposes in the hot loop.
  - v gets a ones column appended: AV lhsT = [v | 1] gives softmax row sums
    in partition row 64 of the o^T PSUM tile for free.
  - sums are batch-transposed (one small PE transpose per 128 t-columns)
    into per-partition layout, reciprocal'd once, and applied per head
    during the projection accumulation (scalar_tensor_tensor).
  - causality at 128-block granularity: s-tile j of t-chunk c only computes
    columns t >= 128j; the diagonal 128x128 triangle gets an additive -1e30
    mask before exp.
"""

import numpy as np

import concourse.bacc as bacc
import concourse.mybir as mybir
import concourse.tile as tile
from concourse.bass_utils import run_bass_kernel_spmd

P = 128
TCH = 512  # t-chunk (one PSUM bank of fp32)
NEG = -1e30
B, T, C, H, D = 4, 2048, 64, 6, 64
HPC = 3  # heads per core
NCORES = 8
F32 = mybir.dt.float32


def build_program(T=T, C=C, D=D, HPC=HPC, mm_dt=mybir.dt.float32r, av_dt=mybir.dt.float32r):
    assert T % TCH == 0
    NCH = T // TCH        # t-chunks of 512
    NB = T // P           # 128-blocks
    BPC = TCH // P        # 128-blocks per chunk (4)
    NSIG = NB             # output row-slices of 128
    nc = bacc.Bacc()

    x_d = nc.dram_tensor("x_loc", [T, C], F32, kind="ExternalInput")
    y_d = nc.dram_tensor("y_loc", [T, C], F32, kind="ExternalInput")
    wq_d = nc.dram_tensor("wq_loc", [HPC, C, D], F32, kind="ExternalInput")
    wk_d = nc.dram_tensor("wk_loc", [HPC, C, D], F32, kind="ExternalInput")
    wv_d = nc.dram_tensor("wv_loc", [HPC, C, D], F32, kind="ExternalInput")
    wp_d = nc.dram_tensor("wp_loc", [HPC, D, C], F32, kind="ExternalInput")
    bias_d = nc.dram_tensor("bias_loc", [1, C], F32, kind="ExternalInput")
    tri_d = nc.dram_tensor("tri_loc", [P, P], F32, kind="ExternalInput")
    ident_d = nc.dram_tensor("ident_loc", [P, P], F32, kind="ExternalInput")
    out_d = nc.dram_tensor("out_loc", [T, C], F32, kind="ExternalOutput")

    scale = float(D) ** -0.5
    NSR = HPC * NCH  # rows in the sums pack tile

    with tile.TileContext(nc) as tc:
        with (
            tc.tile_pool(name="const", bufs=1) as const,
            tc.tile_pool(name="big", bufs=1) as big,
            tc.tile_pool(name="work", bufs=6) as work,
            tc.tile_pool(name="ps_s", bufs=4, space="PSUM") as ps_s,
            tc.tile_pool(name="ps_o", bufs=2, space="PSUM") as ps_o,
            tc.tile_pool(name="ps_m", bufs=2, space="PSUM") as ps_m,
        ):
            # ---- constants -------------------------------------------------
            ident = const.tile([P, P], F32)
            nc.sync.dma_start(out=ident[:, :], in_=ident_d[:, :])
            tri = const.tile([P, P], F32)
            nc.sync.dma_start(out=tri[:, :], in_=tri_d[:, :])
            bias_row = const.tile([1, C], F32)
            nc.sync.dma_start(out=bias_row[:, :], in_=bias_d[:, :])
            bias = const.tile([P, C], F32)
            nc.gpsimd.partition_broadcast(bias[:, :], bias_row[:, :])
            wq_f = const.tile([C, HPC, D], F32)
            nc.sync.dma_start(out=wq_f[:], in_=wq_d[:].rearrange("h c d -> c h d"))
            wk_f = const.tile([C, HPC, D], F32)
            nc.sync.dma_start(out=wk_f[:], in_=wk_d[:].rearrange("h c d -> c h d"))
            wv_f = const.tile([C, HPC, D], F32)
            nc.sync.dma_start(out=wv_f[:], in_=wv_d[:].rearrange("h c d -> c h d"))
            wp_f = const.tile([D, HPC, C], F32)
            nc.sync.dma_start(out=wp_f[:], in_=wp_d[:].rearrange("h d c -> d h c"))
            wq_sb = const.tile([C, HPC, D], mm_dt)
            nc.vector.tensor_copy(wq_sb[:], wq_f[:])
            wk_sb = const.tile([C, HPC, D], mm_dt)
            nc.vector.tensor_copy(wk_sb[:], wk_f[:])
            wv_sb = const.tile([C, HPC, D], mm_dt)
            nc.vector.tensor_copy(wv_sb[:], wv_f[:])
            wp_sb = const.tile([D, HPC, C], mm_dt)
            nc.vector.tensor_copy(wp_sb[:], wp_f[:])

            x_sb = big.tile([P, NB, C], F32)
            nc.sync.dma_start(out=x_sb[:], in_=x_d[:].rearrange("(n p) c -> p n c", p=P))
            y_sb = big.tile([P, NB, C], F32)
            nc.sync.dma_start(out=y_sb[:], in_=y_d[:].rearrange("(n p) c -> p n c", p=P))

            # ---- x^T, y^T via PE transpose --------------------------------
            xT = big.tile([C, T], mm_dt)
            yT = big.tile([C, T], mm_dt)
            for src, dst in ((x_sb, xT), (y_sb, yT)):
                for j in range(NB):
                    tp = ps_m.tile([C, P], F32, tag="mps")
                    nc.tensor.transpose(tp[:, :], src[:, j, :], ident[:, :])
                    nc.vector.tensor_copy(dst[:, j * P : (j + 1) * P], tp[:, :])

            # ---- q^T, k^T, v ----------------------------------------------
            # stacked pair layouts: partitions 0-63 hold q/k for even
            # s-blocks (row group 0), 64-127 for odd s-blocks (row group 64),
            # enabling concurrent S matmuls on both PE row halves.
            qT2 = big.tile([2 * C, HPC, T], mm_dt)
            kT2 = big.tile([2 * C, HPC, (NB // 2) * P], mm_dt)
            v_sb = big.tile([P, HPC, NB, D + 1], av_dt)
            ones_col = const.tile([P, 1], F32)
            nc.vector.memset(ones_col[:, :], 1.0)
            nc.vector.tensor_copy(
                v_sb[:, :, :, D : D + 1],
                ones_col[:, :].to_broadcast((P, HPC, NB, 1)),
            )
            oT_sb = big.tile([C, HPC, T], mm_dt)
            SPP = 32 * (NCH - 1) + 1
            spacks = [big.tile([SPP, TCH], F32, name=f"spack{h}") for h in range(HPC)]
            for h in range(HPC):
                for n4 in range(T // TCH):
                    sl = slice(n4 * TCH, (n4 + 1) * TCH)
                    qp = ps_m.tile([C, TCH], F32, tag="mps")
                    nc.tensor.matmul(
                        qp[:, :], lhsT=wq_sb[:, h, :],
                        rhs=xT[:, sl], start=True, stop=True,
                    )
                    nc.vector.tensor_copy(qT2[0:C, h, sl], qp[:, :])
                    nc.vector.tensor_copy(qT2[C : 2 * C, h, sl], qp[:, :])
                    kp = ps_m.tile([C, TCH], F32, tag="mps")
                    nc.tensor.matmul(
                        kp[:, :], lhsT=wk_sb[:, h, :],
                        rhs=yT[:, sl], start=True, stop=True,
                    )
                    kpv = kp[:, :].rearrange("p (a b) -> p a b", b=P)
                    kdst = kT2[:, h, 2 * n4 * P : (2 * n4 + 2) * P].rearrange(
                        "p (a b) -> p a b", b=P
                    )
                    nc.vector.tensor_copy(kdst[0:C], kpv[:, 0:4:2, :])
                    nc.vector.tensor_copy(kdst[C : 2 * C], kpv[:, 1:4:2, :])
                for g in range(NB // 8):
                    vp = ps_m.tile([P, 8 * D], F32, tag="mps")
                    for jj in range(8):
                        j = g * 8 + jj
                        nc.tensor.matmul(
                            vp[:, jj * D : (jj + 1) * D],
                            lhsT=yT[:, j * P : (j + 1) * P],
                            rhs=wv_sb[:, h, :], start=True, stop=True,
                        )
                    nc.vector.tensor_copy(
                        v_sb[:, h, g * 8 : (g + 1) * 8, 0:D],
                        vp[:, :].rearrange("p (a b) -> p a b", b=D),
                    )

            # ---- attention (emitted per head, interleaved with qkv above
            # via the shared h loop so the scheduler overlaps phases) -------
                for c in range(NCH):
                    ns = BPC * c + BPC
                    ops = ps_o.tile([D + 1, TCH], F32, tag="oT")
                    for pr in range(ns // 2):
                        jA, jB = 2 * pr, 2 * pr + 1
                        sub = []
                        for half, j in ((0, jA), (1, jB)):
                            t0 = max(0, P * j - TCH * c)
                            sps = ps_s.tile([P, TCH], F32, tag="S", name=f"sps{half}")
                            nc.tensor.matmul(
                                sps[:, t0:TCH],
                                lhsT=kT2[half * C : (half + 1) * C, h,
                                         pr * P : (pr + 1) * P],
                                rhs=qT2[half * C : (half + 1) * C, h,
                                        c * TCH + t0 : (c + 1) * TCH],
                                start=True, stop=True,
                            )
                            sub.append((j, t0, sps))
                        for j, t0, sps in sub:
                            if j >= BPC * c:  # diagonal tile: mask the triangle
                                nc.vector.tensor_add(
                                    sps[:, t0 : t0 + P], sps[:, t0 : t0 + P], tri[:, :]
                                )
                            es = work.tile([P, TCH], av_dt, tag="expS")
                            nc.scalar.activation(
                                es[:, t0:TCH], sps[:, t0:TCH],
                                mybir.ActivationFunctionType.Exp, scale=scale,
                            )
                            nc.tensor.matmul(
                                ops[:, t0:TCH],
                                lhsT=v_sb[:, h, j, :],
                                rhs=es[:, t0:TCH],
                                start=(j == 0), stop=(j == ns - 1),
                                skip_group_check=True,
                            )
                    nc.vector.tensor_copy(
                        oT_sb[:, h, c * TCH : (c + 1) * TCH], ops[0:C, :]
                    )
                    nc.vector.tensor_copy(
                        spacks[h][32 * c : 32 * c + 1, :], ops[D : D + 1, :]
                    )

            # ---- reciprocal of softmax sums -------------------------------
            # transpose each head's [97, 512] sums pack (valid rows 32c) into
            # [128, 97] psum blocks; reciprocal only the valid strided columns.
            recip = big.tile([P, HPC, BPC, NCH], F32)
            for h in range(HPC):
                rp = ps_s.tile([P, TCH], F32, tag="S", name=f"rps{h}")
                for sc in range(BPC):
                    nc.tensor.transpose(
                        rp[:, sc * P : sc * P + SPP],
                        spacks[h][0:SPP, sc * P : (sc + 1) * P],
                        ident[0:SPP, 0:SPP],
                    )
                rp_valid = rp[:, :].rearrange("p (a b) -> p a b", a=BPC)[:, :, 0 : 32 * NCH : 32]
                nc.vector.reciprocal(recip[:, h, :, :], rp_valid)

            # ---- projection + normalization + bias ------------------------
            out_full = big.tile([P, NSIG, C], F32)
            for sg in range(NSIG):
                c, sc = sg // BPC, sg % BPC
                pp = ps_m.tile([P, HPC * C], F32, tag="mps")
                for h in range(HPC):
                    nc.tensor.matmul(
                        pp[:, h * C : (h + 1) * C],
                        lhsT=oT_sb[:, h, sg * P : (sg + 1) * P],
                        rhs=wp_sb[:, h, :], start=True, stop=True,
                    )
                acc = bias[:, :]
                for h in range(HPC):
                    dst = (
                        out_full[:, sg, :]
                        if h == HPC - 1
                        else work.tile([P, C], F32, tag="acc", name="acc")[:, :]
                    )
                    nc.vector.scalar_tensor_tensor(
                        out=dst,
                        in0=pp[:, h * C : (h + 1) * C],
                        scalar=recip[:, h, sc, c : c + 1],
                        in1=acc,
                        op0=mybir.AluOpType.mult,
                        op1=mybir.AluOpType.add,
                    )
                    acc = dst
            nc.sync.dma_start(
                out=out_d[:].rearrange("(n p) c -> p n c", p=P), in_=out_full[:]
            )

    nc.compile()
    return nc


_prog = None


def _get_program():
    global _prog
    if _prog is None:
        _prog = build_program()
    return _prog


def make_host_consts():
    i = np.arange(P)
    # tri[s_rel, t_rel]: valid (0) when t_rel >= s_rel else -1e30
    tri = np.where(i[None, :] >= i[:, None], 0.0, NEG).astype(np.float32)
    ident = np.eye(P, dtype=np.float32)
    return tri, ident


def make_in_maps(inputs=None, **kw):
    if inputs is None:
        inputs = kw
    x = np.ascontiguousarray(np.asarray(inputs["x"], np.float32))
    y = np.ascontiguousarray(np.asarray(inputs["y"], np.float32))
    Wq = np.ascontiguousarray(np.asarray(inputs["Wq"], np.float32))
    Wk = np.ascontiguousarray(np.asarray(inputs["Wk"], np.float32))
    Wv = np.ascontiguousarray(np.asarray(inputs["Wv"], np.float32))
    Wp = np.ascontiguousarray(np.asarray(inputs["W_proj"], np.float32)).reshape(
        H, D, C
    )
    b_proj = np.asarray(inputs["b_proj"], np.float32)
    tri, ident = make_host_consts()
    zeros_c = np.zeros_like(b_proj)

    in_maps = []
    for core in range(NCORES):
        b, half = core // 2, core % 2
        h0 = HPC * half
        in_maps.append(
            {
                "x_loc": x[b],
                "y_loc": y[b],
                "wq_loc": Wq[h0 : h0 + HPC],
                "wk_loc": Wk[h0 : h0 + HPC],
                "wv_loc": Wv[h0 : h0 + HPC],
                "wp_loc": Wp[h0 : h0 + HPC],
                "bias_loc": (b_proj if half == 0 else zeros_c)[None, :],
                "tri_loc": tri,
                "ident_loc": ident,
            }
        )
    return in_maps


def assemble(results):
    out = np.stack(
        [results[2 * b]["out_loc"] + results[2 * b + 1]["out_loc"] for b in range(B)]
    )
    return out.astype(np.float32)


def kernel(x, y, Wq, Wk, Wv, W_proj, b_proj):
    in_maps = make_in_maps(
        x=x, y=y, Wq=Wq, Wk=Wk, Wv=Wv, W_proj=W_proj, b_proj=b_proj
    )
    nc = _get_program()
    res = run_bass_kernel_spmd(nc, in_maps, list(range(NCORES))).results
    return assemble(res)


# revision 16
# speedup vs baseline: 1.1027x; 1.0151x over previous
"""Trainium2 Bass kernel: MultiHeadCrossAttention (B=4, T=2048, C=64, H=6, D=64).

Sharding: 8 cores = 4 batches x 2 head-groups of 3. Every core runs an
identical (SPMD) program: causal flash-attention for 3 heads over the full
T=2048 sequence of one batch, followed by its partial output projection.
The two half-head partials per batch are summed on the host during gather.

Layout strategy (all matmuls contract over the partition dim, out = lhsT.T @ rhs):
  - x^T, y^T built once via PE transpose -> q^T,k^T = W.T @ x^T (K=C=64, N=512)
  - scores are computed TRANSPOSED: S^T[s,t] = k^T.T @ q^T  (K=D=64)
    so the AV matmul consumes exp(S^T) directly as its moving operand
    with K=s=128 (full PE) and no transposes in the hot loop.
  - v gets a ones column appended: AV lhsT = [v | 1] gives softmax row sums
    in partition row 64 of the o^T PSUM tile for free.
  - sums are batch-transposed (one small PE transpose per 128 t-columns)
    into per-partition layout, reciprocal'd once, and applied per head
    during the projection accumulation (scalar_tensor_tensor).
  - causality at 128-block granularity: s-tile j of t-chunk c only computes
    columns t >= 128j; the diagonal 128x128 triangle gets an additive -1e30
    mask before exp.
"""

import numpy as np

import concourse.bacc as bacc
import concourse.mybir as mybir
import concourse.tile as tile
from concourse.bass_utils import run_bass_kernel_spmd

P = 128
TCH = 512  # t-chunk (one PSUM bank of fp32)
NEG = -1e30
B, T, C, H, D = 4, 2048, 64, 6, 64
HPC = 3  # heads per core
NCORES = 8
F32 = mybir.dt.float32


def build_program(T=T, C=C, D=D, HPC=HPC, mm_dt=mybir.dt.float32r, av_dt=mybir.dt.float32r):
    assert T % TCH == 0
    NCH = T // TCH        # t-chunks of 512
    NB = T // P           # 128-blocks
    BPC = TCH // P        # 128-blocks per chunk (4)
    NSIG = NB             # output row-slices of 128
    nc = bacc.Bacc()

    x_d = nc.dram_tensor("x_loc", [T, C], F32, kind="ExternalInput")
    y_d = nc.dram_tensor("y_loc", [T, C], F32, kind="ExternalInput")
    wq_d = nc.dram_tensor("wq_loc", [HPC, C, D], F32, kind="ExternalInput")
    wk_d = nc.dram_tensor("wk_loc", [HPC, C, D], F32, kind="ExternalInput")
    wv_d = nc.dram_tensor("wv_loc", [HPC, C, D], F32, kind="ExternalInput")
    wp_d = nc.dram_tensor("wp_loc", [HPC, D, C], F32, kind="ExternalInput")
    bias_d = nc.dram_tensor("bias_loc", [1, C], F32, kind="ExternalInput")
    tri_d = nc.dram_tensor("tri_loc", [P, P], F32, kind="ExternalInput")
    ident_d = nc.dram_tensor("ident_loc", [P, P], F32, kind="ExternalInput")
    out_d = nc.dram_tensor("out_loc", [T, C], F32, kind="ExternalOutput")

    scale = float(D) ** -0.5
    NSR = HPC * NCH  # rows in the sums pack tile

    with tile.TileContext(nc) as tc:
        with (
            tc.tile_pool(name="const", bufs=1) as const,
            tc.tile_pool(name="big", bufs=1) as big,
            tc.tile_pool(name="work", bufs=6) as work,
            tc.tile_pool(name="ps_s", bufs=4, space="PSUM") as ps_s,
            tc.tile_pool(name="ps_o", bufs=2, space="PSUM") as ps_o,
            tc.tile_pool(name="ps_m", bufs=2, space="PSUM") as ps_m,
        ):
            # ---- constants -------------------------------------------------
            ident = const.tile([P, P], F32)
            nc.sync.dma_start(out=ident[:, :], in_=ident_d[:, :])
            tri = const.tile([P, P], F32)
            nc.sync.dma_start(out=tri[:, :], in_=tri_d[:, :])
            bias_row = const.tile([1, C], F32)
            nc.sync.dma_start(out=bias_row[:, :], in_=bias_d[:, :])
            bias = const.tile([P, C], F32)
            nc.gpsimd.partition_broadcast(bias[:, :], bias_row[:, :])
            wq_f = const.tile([C, HPC, D], F32)
            nc.sync.dma_start(out=wq_f[:], in_=wq_d[:].rearrange("h c d -> c h d"))
            wk_f = const.tile([C, HPC, D], F32)
            nc.sync.dma_start(out=wk_f[:], in_=wk_d[:].rearrange("h c d -> c h d"))
            wv_f = const.tile([C, HPC, D], F32)
            nc.sync.dma_start(out=wv_f[:], in_=wv_d[:].rearrange("h c d -> c h d"))
            wp_f = const.tile([D, HPC, C], F32)
            nc.sync.dma_start(out=wp_f[:], in_=wp_d[:].rearrange("h d c -> d h c"))
            wq_sb = const.tile([C, HPC, D], mm_dt)
            nc.vector.tensor_copy(wq_sb[:], wq_f[:])
            wk_sb = const.tile([C, HPC, D], mm_dt)
            nc.vector.tensor_copy(wk_sb[:], wk_f[:])
            wv_sb = const.tile([C, HPC, D], mm_dt)
            nc.vector.tensor_copy(wv_sb[:], wv_f[:])
            wp_sb = const.tile([D, HPC, C], mm_dt)
            nc.vector.tensor_copy(wp_sb[:], wp_f[:])

            x_sb = big.tile([P, NB, C], F32)
            y_sb = big.tile([P, NB, C], F32)
            QC = NB // 4  # DMA chunk: 4 blocks
            xv = x_d[:].rearrange("(n p) c -> p n c", p=P)
            yv = y_d[:].rearrange("(n p) c -> p n c", p=P)
            for g in range(4):
                sl = slice(g * QC, (g + 1) * QC)
                nc.sync.dma_start(out=y_sb[:, sl, :], in_=yv[:, sl, :])
            for g in range(4):
                sl = slice(g * QC, (g + 1) * QC)
                nc.sync.dma_start(out=x_sb[:, sl, :], in_=xv[:, sl, :])

            # ---- x^T, y^T via PE transpose (y first: it gates k/v and S) --
            xT = big.tile([C, T], mm_dt)
            yT = big.tile([C, T], mm_dt)
            for src, dst in ((y_sb, yT), (x_sb, xT)):
                for j in range(NB):
                    tp = ps_m.tile([C, P], F32, tag="mps")
                    nc.tensor.transpose(tp[:, :], src[:, j, :], ident[:, :])
                    nc.vector.tensor_copy(dst[:, j * P : (j + 1) * P], tp[:, :])

            # ---- q^T, k^T, v ----------------------------------------------
            # stacked pair layouts: partitions 0-63 hold q/k for even
            # s-blocks (row group 0), 64-127 for odd s-blocks (row group 64),
            # enabling concurrent S matmuls on both PE row halves.
            qT2 = big.tile([2 * C, HPC, T], mm_dt)
            kT2 = big.tile([2 * C, HPC, (NB // 2) * P], mm_dt)
            v_sb = big.tile([P, HPC, NB, D + 1], av_dt)
            ones_col = const.tile([P, 1], F32)
            nc.vector.memset(ones_col[:, :], 1.0)
            nc.vector.tensor_copy(
                v_sb[:, :, :, D : D + 1],
                ones_col[:, :].to_broadcast((P, HPC, NB, 1)),
            )
            oT_sb = big.tile([C, HPC, T], mm_dt)
            SPP = 32 * (NCH - 1) + 1
            spacks = [big.tile([SPP, TCH], F32, name=f"spack{h}") for h in range(HPC)]
            for h in range(HPC):
                for n4 in range(T // TCH):
                    sl = slice(n4 * TCH, (n4 + 1) * TCH)
                    qp = ps_m.tile([C, TCH], F32, tag="mps")
                    nc.tensor.matmul(
                        qp[:, :], lhsT=wq_sb[:, h, :],
                        rhs=xT[:, sl], start=True, stop=True,
                    )
                    nc.vector.tensor_copy(qT2[0:C, h, sl], qp[:, :])
                    nc.vector.tensor_copy(qT2[C : 2 * C, h, sl], qp[:, :])
                    kp = ps_m.tile([C, TCH], F32, tag="mps")
                    nc.tensor.matmul(
                        kp[:, :], lhsT=wk_sb[:, h, :],
                        rhs=yT[:, sl], start=True, stop=True,
                    )
                    kpv = kp[:, :].rearrange("p (a b) -> p a b", b=P)
                    kdst = kT2[:, h, 2 * n4 * P : (2 * n4 + 2) * P].rearrange(
                        "p (a b) -> p a b", b=P
                    )
                    nc.vector.tensor_copy(kdst[0:C], kpv[:, 0:4:2, :])
                    nc.vector.tensor_copy(kdst[C : 2 * C], kpv[:, 1:4:2, :])
                for g in range(NB // 8):
                    vp = ps_m.tile([P, 8 * D], F32, tag="mps")
                    for jj in range(8):
                        j = g * 8 + jj
                        nc.tensor.matmul(
                            vp[:, jj * D : (jj + 1) * D],
                            lhsT=yT[:, j * P : (j + 1) * P],
                            rhs=wv_sb[:, h, :], start=True, stop=True,
                        )
                    nc.vector.tensor_copy(
                        v_sb[:, h, g * 8 : (g + 1) * 8, 0:D],
                        vp[:, :].rearrange("p (a b) -> p a b", b=D),
                    )

            # ---- attention (emitted per head, interleaved with qkv above
            # via the shared h loop so the scheduler overlaps phases) -------
                for c in range(NCH):
                    ns = BPC * c + BPC
                    ops = ps_o.tile([D + 1, TCH], F32, tag="oT")
                    for pr in range(ns // 2):
                        jA, jB = 2 * pr, 2 * pr + 1
                        sub = []
                        for half, j in ((0, jA), (1, jB)):
                            t0 = max(0, P * j - TCH * c)
                            sps = ps_s.tile([P, TCH], F32, tag="S", name=f"sps{half}")
                            nc.tensor.matmul(
                                sps[:, t0:TCH],
                                lhsT=kT2[half * C : (half + 1) * C, h,
                                         pr * P : (pr + 1) * P],
                                rhs=qT2[half * C : (half + 1) * C, h,
                                        c * TCH + t0 : (c + 1) * TCH],
                                start=True, stop=True,
                            )
                            sub.append((j, t0, sps))
                        for j, t0, sps in sub:
                            if j >= BPC * c:  # diagonal tile: mask the triangle
                                nc.vector.tensor_add(
                                    sps[:, t0 : t0 + P], sps[:, t0 : t0 + P], tri[:, :]
                                )
                            es = work.tile([P, TCH], av_dt, tag="expS")
                            nc.scalar.activation(
                                es[:, t0:TCH], sps[:, t0:TCH],
                                mybir.ActivationFunctionType.Exp, scale=scale,
                            )
                            nc.tensor.matmul(
                                ops[:, t0:TCH],
                                lhsT=v_sb[:, h, j, :],
                                rhs=es[:, t0:TCH],
                                start=(j == 0), stop=(j == ns - 1),
                                skip_group_check=True,
                            )
                    nc.vector.tensor_copy(
                        oT_sb[:, h, c * TCH : (c + 1) * TCH], ops[0:C, :]
                    )
                    nc.vector.tensor_copy(
                        spacks[h][32 * c : 32 * c + 1, :], ops[D : D + 1, :]
                    )

            # ---- reciprocal of softmax sums -------------------------------
            # transpose each head's [97, 512] sums pack (valid rows 32c) into
            # [128, 97] psum blocks; reciprocal only the valid strided columns.
            recip = big.tile([P, HPC, BPC, NCH], F32)
            for h in range(HPC):
                rp = ps_s.tile([P, TCH], F32, tag="S", name=f"rps{h}")
                for sc in range(BPC):
                    nc.tensor.transpose(
                        rp[:, sc * P : sc * P + SPP],
                        spacks[h][0:SPP, sc * P : (sc + 1) * P],
                        ident[0:SPP, 0:SPP],
                    )
                rp_valid = rp[:, :].rearrange("p (a b) -> p a b", a=BPC)[:, :, 0 : 32 * NCH : 32]
                nc.vector.reciprocal(recip[:, h, :, :], rp_valid)

            # ---- projection + normalization + bias ------------------------
            out_full = big.tile([P, NSIG, C], F32)
            for sg in range(NSIG):
                c, sc = sg // BPC, sg % BPC
                pp = ps_m.tile([P, HPC * C], F32, tag="mps")
                for h in range(HPC):
                    nc.tensor.matmul(
                        pp[:, h * C : (h + 1) * C],
                        lhsT=oT_sb[:, h, sg * P : (sg + 1) * P],
                        rhs=wp_sb[:, h, :], start=True, stop=True,
                    )
                acc = bias[:, :]
                for h in range(HPC):
                    dst = (
                        out_full[:, sg, :]
                        if h == HPC - 1
                        else work.tile([P, C], F32, tag="acc", name="acc")[:, :]
                    )
                    nc.vector.scalar_tensor_tensor(
                        out=dst,
                        in0=pp[:, h * C : (h + 1) * C],
                        scalar=recip[:, h, sc, c : c + 1],
                        in1=acc,
                        op0=mybir.AluOpType.mult,
                        op1=mybir.AluOpType.add,
                    )
                    acc = dst
            outv = out_d[:].rearrange("(n p) c -> p n c", p=P)
            for g in range(4):
                sl = slice(g * (NSIG // 4), (g + 1) * (NSIG // 4))
                nc.sync.dma_start(out=outv[:, sl, :], in_=out_full[:, sl, :])

    nc.compile()
    return nc


_prog = None


def _get_program():
    global _prog
    if _prog is None:
        _prog = build_program()
    return _prog


def make_host_consts():
    i = np.arange(P)
    # tri[s_rel, t_rel]: valid (0) when t_rel >= s_rel else -1e30
    tri = np.where(i[None, :] >= i[:, None], 0.0, NEG).astype(np.float32)
    ident = np.eye(P, dtype=np.float32)
    return tri, ident


def make_in_maps(inputs=None, **kw):
    if inputs is None:
        inputs = kw
    x = np.ascontiguousarray(np.asarray(inputs["x"], np.float32))
    y = np.ascontiguousarray(np.asarray(inputs["y"], np.float32))
    Wq = np.ascontiguousarray(np.asarray(inputs["Wq"], np.float32))
    Wk = np.ascontiguousarray(np.asarray(inputs["Wk"], np.float32))
    Wv = np.ascontiguousarray(np.asarray(inputs["Wv"], np.float32))
    Wp = np.ascontiguousarray(np.asarray(inputs["W_proj"], np.float32)).reshape(
        H, D, C
    )
    b_proj = np.asarray(inputs["b_proj"], np.float32)
    tri, ident = make_host_consts()
    zeros_c = np.zeros_like(b_proj)

    in_maps = []
    for core in range(NCORES):
        b, half = core // 2, core % 2
        h0 = HPC * half
        in_maps.append(
            {
                "x_loc": x[b],
                "y_loc": y[b],
                "wq_loc": Wq[h0 : h0 + HPC],
                "wk_loc": Wk[h0 : h0 + HPC],
                "wv_loc": Wv[h0 : h0 + HPC],
                "wp_loc": Wp[h0 : h0 + HPC],
                "bias_loc": (b_proj if half == 0 else zeros_c)[None, :],
                "tri_loc": tri,
                "ident_loc": ident,
            }
        )
    return in_maps


def assemble(results):
    out = np.stack(
        [results[2 * b]["out_loc"] + results[2 * b + 1]["out_loc"] for b in range(B)]
    )
    return out.astype(np.float32)


def kernel(x, y, Wq, Wk, Wv, W_proj, b_proj):
    in_maps = make_in_maps(
        x=x, y=y, Wq=Wq, Wk=Wk, Wv=Wv, W_proj=W_proj, b_proj=b_proj
    )
    nc = _get_program()
    res = run_bass_kernel_spmd(nc, in_maps, list(range(NCORES))).results
    return assemble(res)


# revision 17
# speedup vs baseline: 1.1294x; 1.0243x over previous
"""Trainium2 Bass kernel: MultiHeadCrossAttention (B=4, T=2048, C=64, H=6, D=64).

Sharding: 8 cores = 4 batches x 2 head-groups of 3. Every core runs an
identical (SPMD) program: causal flash-attention for 3 heads over the full
T=2048 sequence of one batch, followed by its partial output projection.
The two half-head partials per batch are summed on the host during gather.

Layout strategy (all matmuls contract over the partition dim, out = lhsT.T @ rhs):
  - x^T, y^T built once via PE transpose -> q^T,k^T = W.T @ x^T (K=C=64, N=512)
  - scores are computed TRANSPOSED: S^T[s,t] = k^T.T @ q^T  (K=D=64)
    so the AV matmul consumes exp(S^T) directly as its moving operand
    with K=s=128 (full PE) and no transposes in the hot loop.
  - v gets a ones column appended: AV lhsT = [v | 1] gives softmax row sums
    in partition row 64 of the o^T PSUM tile for free.
  - sums are batch-transposed (one small PE transpose per 128 t-columns)
    into per-partition layout, reciprocal'd once, and applied per head
    during the projection accumulation (scalar_tensor_tensor).
  - causality at 128-block granularity: s-tile j of t-chunk c only computes
    columns t >= 128j; the diagonal 128x128 triangle gets an additive -1e30
    mask before exp.
"""

import numpy as np

import concourse.bacc as bacc
import concourse.mybir as mybir
import concourse.tile as tile
from concourse.bass_utils import run_bass_kernel_spmd

P = 128
TCH = 512  # t-chunk (one PSUM bank of fp32)
NEG = -1e30
B, T, C, H, D = 4, 2048, 64, 6, 64
HPC = 3  # heads per core
NCORES = 8
F32 = mybir.dt.float32


def build_program(T=T, C=C, D=D, HPC=HPC, mm_dt=mybir.dt.float32r, av_dt=mybir.dt.float32r):
    assert T % TCH == 0
    NCH = T // TCH        # t-chunks of 512
    NB = T // P           # 128-blocks
    BPC = TCH // P        # 128-blocks per chunk (4)
    NSIG = NB             # output row-slices of 128
    nc = bacc.Bacc()

    x_d = nc.dram_tensor("x_loc", [T, C], F32, kind="ExternalInput")
    y_d = nc.dram_tensor("y_loc", [T, C], F32, kind="ExternalInput")
    wq_d = nc.dram_tensor("wq_loc", [HPC, C, D], F32, kind="ExternalInput")
    wk_d = nc.dram_tensor("wk_loc", [HPC, C, D], F32, kind="ExternalInput")
    wv_d = nc.dram_tensor("wv_loc", [HPC, C, D], F32, kind="ExternalInput")
    wp_d = nc.dram_tensor("wp_loc", [HPC, D, C], F32, kind="ExternalInput")
    bias_d = nc.dram_tensor("bias_loc", [1, C], F32, kind="ExternalInput")
    tri_d = nc.dram_tensor("tri_loc", [P, P], F32, kind="ExternalInput")
    ident_d = nc.dram_tensor("ident_loc", [P, P], F32, kind="ExternalInput")
    out_d = nc.dram_tensor("out_loc", [T, C], F32, kind="ExternalOutput")

    scale = float(D) ** -0.5
    NSR = HPC * NCH  # rows in the sums pack tile

    with tile.TileContext(nc) as tc:
        with (
            tc.tile_pool(name="const", bufs=1) as const,
            tc.tile_pool(name="big", bufs=1) as big,
            tc.tile_pool(name="work", bufs=6) as work,
            tc.tile_pool(name="ps_s", bufs=3, space="PSUM") as ps_s,
            tc.tile_pool(name="ps_o", bufs=2, space="PSUM") as ps_o,
            tc.tile_pool(name="ps_m", bufs=3, space="PSUM") as ps_m,
        ):
            # ---- constants -------------------------------------------------
            ident = const.tile([P, P], F32)
            nc.sync.dma_start(out=ident[:, :], in_=ident_d[:, :])
            tri = const.tile([P, P], F32)
            nc.sync.dma_start(out=tri[:, :], in_=tri_d[:, :])
            bias_row = const.tile([1, C], F32)
            nc.sync.dma_start(out=bias_row[:, :], in_=bias_d[:, :])
            bias = const.tile([P, C], F32)
            nc.gpsimd.partition_broadcast(bias[:, :], bias_row[:, :])
            wq_f = const.tile([C, HPC, D], F32)
            nc.sync.dma_start(out=wq_f[:], in_=wq_d[:].rearrange("h c d -> c h d"))
            wk_f = const.tile([C, HPC, D], F32)
            nc.sync.dma_start(out=wk_f[:], in_=wk_d[:].rearrange("h c d -> c h d"))
            wv_f = const.tile([C, HPC, D], F32)
            nc.sync.dma_start(out=wv_f[:], in_=wv_d[:].rearrange("h c d -> c h d"))
            wp_f = const.tile([D, HPC, C], F32)
            nc.sync.dma_start(out=wp_f[:], in_=wp_d[:].rearrange("h d c -> d h c"))
            wq_sb = const.tile([C, HPC, D], mm_dt)
            nc.vector.tensor_copy(wq_sb[:], wq_f[:])
            wk_sb = const.tile([C, HPC, D], mm_dt)
            nc.vector.tensor_copy(wk_sb[:], wk_f[:])
            wv_sb = const.tile([C, HPC, D], mm_dt)
            nc.vector.tensor_copy(wv_sb[:], wv_f[:])
            wp_sb = const.tile([D, HPC, C], mm_dt)
            nc.vector.tensor_copy(wp_sb[:], wp_f[:])

            x_sb = big.tile([P, NB, C], F32)
            y_sb = big.tile([P, NB, C], F32)
            QC = NB // 4  # DMA chunk: 4 blocks
            xv = x_d[:].rearrange("(n p) c -> p n c", p=P)
            yv = y_d[:].rearrange("(n p) c -> p n c", p=P)
            for g in range(4):
                sl = slice(g * QC, (g + 1) * QC)
                nc.sync.dma_start(out=y_sb[:, sl, :], in_=yv[:, sl, :])
            for g in range(4):
                sl = slice(g * QC, (g + 1) * QC)
                nc.sync.dma_start(out=x_sb[:, sl, :], in_=xv[:, sl, :])

            # ---- x^T, y^T via PE transpose (y first: it gates k/v and S) --
            xT = big.tile([C, T], mm_dt)
            yT = big.tile([C, T], mm_dt)
            for src, dst in ((y_sb, yT), (x_sb, xT)):
                for j in range(NB):
                    tp = ps_m.tile([C, P], F32, tag="mps")
                    nc.tensor.transpose(tp[:, :], src[:, j, :], ident[:, :])
                    nc.vector.tensor_copy(dst[:, j * P : (j + 1) * P], tp[:, :])

            # ---- q^T, k^T, v ----------------------------------------------
            # stacked pair layouts: partitions 0-63 hold q/k for even
            # s-blocks (row group 0), 64-127 for odd s-blocks (row group 64),
            # enabling concurrent S matmuls on both PE row halves.
            qT2 = big.tile([2 * C, HPC, T], mm_dt)
            kT2 = big.tile([2 * C, HPC, (NB // 2) * P], mm_dt)
            v_sb = big.tile([P, HPC, NB, D + 1], av_dt)
            ones_col = const.tile([P, 1], F32)
            nc.vector.memset(ones_col[:, :], 1.0)
            nc.vector.tensor_copy(
                v_sb[:, :, :, D : D + 1],
                ones_col[:, :].to_broadcast((P, HPC, NB, 1)),
            )
            oT_sb = big.tile([C, HPC, T], mm_dt)
            SPP = 32 * (NCH - 1) + 1
            spacks = [big.tile([SPP, TCH], F32, name=f"spack{h}") for h in range(HPC)]
            for h in range(HPC):
                for n4 in range(T // TCH):
                    sl = slice(n4 * TCH, (n4 + 1) * TCH)
                    qp = ps_m.tile([C, TCH], F32, tag="mps")
                    nc.tensor.matmul(
                        qp[:, :], lhsT=wq_sb[:, h, :],
                        rhs=xT[:, sl], start=True, stop=True,
                    )
                    nc.vector.tensor_copy(qT2[0:C, h, sl], qp[:, :])
                    nc.vector.tensor_copy(qT2[C : 2 * C, h, sl], qp[:, :])
                    kp = ps_m.tile([C, TCH], F32, tag="mps")
                    nc.tensor.matmul(
                        kp[:, :], lhsT=wk_sb[:, h, :],
                        rhs=yT[:, sl], start=True, stop=True,
                    )
                    kpv = kp[:, :].rearrange("p (a b) -> p a b", b=P)
                    kdst = kT2[:, h, 2 * n4 * P : (2 * n4 + 2) * P].rearrange(
                        "p (a b) -> p a b", b=P
                    )
                    nc.vector.tensor_copy(kdst[0:C], kpv[:, 0:4:2, :])
                    nc.vector.tensor_copy(kdst[C : 2 * C], kpv[:, 1:4:2, :])
                for g in range(NB // 8):
                    vp = ps_m.tile([P, 8 * D], F32, tag="mps")
                    for jj in range(8):
                        j = g * 8 + jj
                        nc.tensor.matmul(
                            vp[:, jj * D : (jj + 1) * D],
                            lhsT=yT[:, j * P : (j + 1) * P],
                            rhs=wv_sb[:, h, :], start=True, stop=True,
                        )
                    nc.vector.tensor_copy(
                        v_sb[:, h, g * 8 : (g + 1) * 8, 0:D],
                        vp[:, :].rearrange("p (a b) -> p a b", b=D),
                    )

            # ---- attention (emitted per head, interleaved with qkv above
            # via the shared h loop so the scheduler overlaps phases) -------
                for c in range(NCH):
                    ns = BPC * c + BPC
                    ops = ps_o.tile([D + 1, TCH], F32, tag="oT")
                    for pr in range(ns // 2):
                        jA, jB = 2 * pr, 2 * pr + 1
                        sub = []
                        for half, j in ((0, jA), (1, jB)):
                            t0 = max(0, P * j - TCH * c)
                            sps = ps_s.tile([P, TCH], F32, tag="S", name=f"sps{half}")
                            nc.tensor.matmul(
                                sps[:, t0:TCH],
                                lhsT=kT2[half * C : (half + 1) * C, h,
                                         pr * P : (pr + 1) * P],
                                rhs=qT2[half * C : (half + 1) * C, h,
                                        c * TCH + t0 : (c + 1) * TCH],
                                start=True, stop=True,
                            )
                            sub.append((j, t0, sps))
                        for j, t0, sps in sub:
                            if j >= BPC * c:  # diagonal tile: mask the triangle
                                nc.vector.tensor_add(
                                    sps[:, t0 : t0 + P], sps[:, t0 : t0 + P], tri[:, :]
                                )
                            es = work.tile([P, TCH], av_dt, tag="expS")
                            nc.scalar.activation(
                                es[:, t0:TCH], sps[:, t0:TCH],
                                mybir.ActivationFunctionType.Exp, scale=scale,
                            )
                            nc.tensor.matmul(
                                ops[:, t0:TCH],
                                lhsT=v_sb[:, h, j, :],
                                rhs=es[:, t0:TCH],
                                start=(j == 0), stop=(j == ns - 1),
                                skip_group_check=True,
                            )
                    nc.vector.tensor_copy(
                        oT_sb[:, h, c * TCH : (c + 1) * TCH], ops[0:C, :]
                    )
                    nc.vector.tensor_copy(
                        spacks[h][32 * c : 32 * c + 1, :], ops[D : D + 1, :]
                    )

            # ---- reciprocal of softmax sums -------------------------------
            # transpose each head's [97, 512] sums pack (valid rows 32c) into
            # [128, 97] psum blocks; reciprocal only the valid strided columns.
            recip = big.tile([P, HPC, BPC, NCH], F32)
            for h in range(HPC):
                rp = ps_s.tile([P, TCH], F32, tag="S", name=f"rps{h}")
                for sc in range(BPC):
                    nc.tensor.transpose(
                        rp[:, sc * P : sc * P + SPP],
                        spacks[h][0:SPP, sc * P : (sc + 1) * P],
                        ident[0:SPP, 0:SPP],
                    )
                rp_valid = rp[:, :].rearrange("p (a b) -> p a b", a=BPC)[:, :, 0 : 32 * NCH : 32]
                nc.vector.reciprocal(recip[:, h, :, :], rp_valid)

            # ---- projection + normalization + bias ------------------------
            out_full = big.tile([P, NSIG, C], F32)
            for sg in range(NSIG):
                c, sc = sg // BPC, sg % BPC
                pp = ps_m.tile([P, HPC * C], F32, tag="mps")
                for h in range(HPC):
                    nc.tensor.matmul(
                        pp[:, h * C : (h + 1) * C],
                        lhsT=oT_sb[:, h, sg * P : (sg + 1) * P],
                        rhs=wp_sb[:, h, :], start=True, stop=True,
                    )
                acc = bias[:, :]
                for h in range(HPC):
                    dst = (
                        out_full[:, sg, :]
                        if h == HPC - 1
                        else work.tile([P, C], F32, tag="acc", name="acc")[:, :]
                    )
                    nc.vector.scalar_tensor_tensor(
                        out=dst,
                        in0=pp[:, h * C : (h + 1) * C],
                        scalar=recip[:, h, sc, c : c + 1],
                        in1=acc,
                        op0=mybir.AluOpType.mult,
                        op1=mybir.AluOpType.add,
                    )
                    acc = dst
            outv = out_d[:].rearrange("(n p) c -> p n c", p=P)
            for g in range(4):
                sl = slice(g * (NSIG // 4), (g + 1) * (NSIG // 4))
                nc.sync.dma_start(out=outv[:, sl, :], in_=out_full[:, sl, :])

    nc.compile()
    return nc


_prog = None


def _get_program():
    global _prog
    if _prog is None:
        _prog = build_program()
    return _prog


def make_host_consts():
    i = np.arange(P)
    # tri[s_rel, t_rel]: valid (0) when t_rel >= s_rel else -1e30
    tri = np.where(i[None, :] >= i[:, None], 0.0, NEG).astype(np.float32)
    ident = np.eye(P, dtype=np.float32)
    return tri, ident


def make_in_maps(inputs=None, **kw):
    if inputs is None:
        inputs = kw
    x = np.ascontiguousarray(np.asarray(inputs["x"], np.float32))
    y = np.ascontiguousarray(np.asarray(inputs["y"], np.float32))
    Wq = np.ascontiguousarray(np.asarray(inputs["Wq"], np.float32))
    Wk = np.ascontiguousarray(np.asarray(inputs["Wk"], np.float32))
    Wv = np.ascontiguousarray(np.asarray(inputs["Wv"], np.float32))
    Wp = np.ascontiguousarray(np.asarray(inputs["W_proj"], np.float32)).reshape(
        H, D, C
    )
    b_proj = np.asarray(inputs["b_proj"], np.float32)
    tri, ident = make_host_consts()
    zeros_c = np.zeros_like(b_proj)

    in_maps = []
    for core in range(NCORES):
        b, half = core // 2, core % 2
        h0 = HPC * half
        in_maps.append(
            {
                "x_loc": x[b],
                "y_loc": y[b],
                "wq_loc": Wq[h0 : h0 + HPC],
                "wk_loc": Wk[h0 : h0 + HPC],
                "wv_loc": Wv[h0 : h0 + HPC],
                "wp_loc": Wp[h0 : h0 + HPC],
                "bias_loc": (b_proj if half == 0 else zeros_c)[None, :],
                "tri_loc": tri,
                "ident_loc": ident,
            }
        )
    return in_maps


def assemble(results):
    out = np.stack(
        [results[2 * b]["out_loc"] + results[2 * b + 1]["out_loc"] for b in range(B)]
    )
    return out.astype(np.float32)


def kernel(x, y, Wq, Wk, Wv, W_proj, b_proj):
    in_maps = make_in_maps(
        x=x, y=y, Wq=Wq, Wk=Wk, Wv=Wv, W_proj=W_proj, b_proj=b_proj
    )
    nc = _get_program()
    res = run_bass_kernel_spmd(nc, in_maps, list(range(NCORES))).results
    return assemble(res)


# revision 19
# speedup vs baseline: 1.1569x; 1.0243x over previous
"""Trainium2 Bass kernel: MultiHeadCrossAttention (B=4, T=2048, C=64, H=6, D=64).

Sharding: 8 cores = 4 batches x 2 head-groups of 3. Every core runs an
identical (SPMD) program: causal flash-attention for 3 heads over the full
T=2048 sequence of one batch, followed by its partial output projection.
The two half-head partials per batch are summed on the host during gather.

Layout strategy (all matmuls contract over the partition dim, out = lhsT.T @ rhs):
  - x^T, y^T built once via PE transpose -> q^T,k^T = W.T @ x^T (K=C=64, N=512)
  - scores are computed TRANSPOSED: S^T[s,t] = k^T.T @ q^T  (K=D=64)
    so the AV matmul consumes exp(S^T) directly as its moving operand
    with K=s=128 (full PE) and no transposes in the hot loop.
  - v gets a ones column appended: AV lhsT = [v | 1] gives softmax row sums
    in partition row 64 of the o^T PSUM tile for free.
  - sums are batch-transposed (one small PE transpose per 128 t-columns)
    into per-partition layout, reciprocal'd once, and applied per head
    during the projection accumulation (scalar_tensor_tensor).
  - causality at 128-block granularity: s-tile j of t-chunk c only computes
    columns t >= 128j; the diagonal 128x128 triangle gets an additive -1e30
    mask before exp.
"""

import numpy as np

import concourse.bacc as bacc
import concourse.mybir as mybir
import concourse.tile as tile
from concourse.bass_utils import run_bass_kernel_spmd

P = 128
TCH = 512  # t-chunk (one PSUM bank of fp32)
NEG = -1e30
B, T, C, H, D = 4, 2048, 64, 6, 64
HPC = 3  # heads per core
NCORES = 8
F32 = mybir.dt.float32


def build_program(T=T, C=C, D=D, HPC=HPC, mm_dt=mybir.dt.float32r, av_dt=mybir.dt.float32r):
    assert T % TCH == 0
    NCH = T // TCH        # t-chunks of 512
    NB = T // P           # 128-blocks
    BPC = TCH // P        # 128-blocks per chunk (4)
    NSIG = NB             # output row-slices of 128
    nc = bacc.Bacc()

    x_d = nc.dram_tensor("x_loc", [T, C], F32, kind="ExternalInput")
    y_d = nc.dram_tensor("y_loc", [T, C], F32, kind="ExternalInput")
    wq_d = nc.dram_tensor("wq_loc", [HPC, C, D], F32, kind="ExternalInput")
    wk_d = nc.dram_tensor("wk_loc", [HPC, C, D], F32, kind="ExternalInput")
    wv_d = nc.dram_tensor("wv_loc", [HPC, C, D], F32, kind="ExternalInput")
    wp_d = nc.dram_tensor("wp_loc", [HPC, D, C], F32, kind="ExternalInput")
    bias_d = nc.dram_tensor("bias_loc", [1, C], F32, kind="ExternalInput")
    tri_d = nc.dram_tensor("tri_loc", [P, P], F32, kind="ExternalInput")
    ident_d = nc.dram_tensor("ident_loc", [P, P], F32, kind="ExternalInput")
    out_d = nc.dram_tensor("out_loc", [T, C], F32, kind="ExternalOutput")

    scale = float(D) ** -0.5
    NSR = HPC * NCH  # rows in the sums pack tile

    with tile.TileContext(nc) as tc:
        with (
            tc.tile_pool(name="const", bufs=1) as const,
            tc.tile_pool(name="big", bufs=1) as big,
            tc.tile_pool(name="work", bufs=6) as work,
            tc.tile_pool(name="ps_s", bufs=3, space="PSUM") as ps_s,
            tc.tile_pool(name="ps_o", bufs=2, space="PSUM") as ps_o,
            tc.tile_pool(name="ps_m", bufs=3, space="PSUM") as ps_m,
        ):
            # ---- constants -------------------------------------------------
            ident = const.tile([P, P], F32)
            nc.sync.dma_start(out=ident[:, :], in_=ident_d[:, :])
            tri = const.tile([P, P], F32)
            nc.sync.dma_start(out=tri[:, :], in_=tri_d[:, :])
            bias_row = const.tile([1, C], F32)
            nc.sync.dma_start(out=bias_row[:, :], in_=bias_d[:, :])
            bias = const.tile([P, C], F32)
            nc.gpsimd.partition_broadcast(bias[:, :], bias_row[:, :])
            wq_f = const.tile([C, HPC, D], F32)
            nc.sync.dma_start(out=wq_f[:], in_=wq_d[:].rearrange("h c d -> c h d"))
            wk_f = const.tile([C, HPC, D], F32)
            nc.sync.dma_start(out=wk_f[:], in_=wk_d[:].rearrange("h c d -> c h d"))
            wv_f = const.tile([C, HPC, D], F32)
            nc.sync.dma_start(out=wv_f[:], in_=wv_d[:].rearrange("h c d -> c h d"))
            wp_f = const.tile([D, HPC, C], F32)
            nc.sync.dma_start(out=wp_f[:], in_=wp_d[:].rearrange("h d c -> d h c"))
            wq_sb = const.tile([C, HPC, D], mm_dt)
            nc.vector.tensor_copy(wq_sb[:], wq_f[:])
            wk_sb = const.tile([C, HPC, D], mm_dt)
            nc.vector.tensor_copy(wk_sb[:], wk_f[:])
            wv_sb = const.tile([C, HPC, D], mm_dt)
            nc.vector.tensor_copy(wv_sb[:], wv_f[:])
            wp_sb = const.tile([D, HPC, C], mm_dt)
            nc.vector.tensor_copy(wp_sb[:], wp_f[:])

            x_sb = big.tile([P, NB, C], F32)
            y_sb = big.tile([P, NB, C], F32)
            QC = NB // 4  # DMA chunk: 4 blocks
            xv = x_d[:].rearrange("(n p) c -> p n c", p=P)
            yv = y_d[:].rearrange("(n p) c -> p n c", p=P)
            for g in range(4):
                sl = slice(g * QC, (g + 1) * QC)
                nc.sync.dma_start(out=y_sb[:, sl, :], in_=yv[:, sl, :])
            for g in range(4):
                sl = slice(g * QC, (g + 1) * QC)
                nc.sync.dma_start(out=x_sb[:, sl, :], in_=xv[:, sl, :])

            # ---- x^T, y^T via PE transpose (y first: it gates k/v and S) --
            xT = big.tile([C, T], mm_dt)
            yT = big.tile([C, T], mm_dt)
            for src, dst in ((y_sb, yT), (x_sb, xT)):
                for j in range(NB):
                    tp = ps_m.tile([C, P], F32, tag="mps")
                    nc.tensor.transpose(tp[:, :], src[:, j, :], ident[:, :])
                    nc.vector.tensor_copy(dst[:, j * P : (j + 1) * P], tp[:, :])

            # ---- q^T, k^T, v ----------------------------------------------
            # stacked pair layouts: partitions 0-63 hold q/k for even
            # s-blocks (row group 0), 64-127 for odd s-blocks (row group 64),
            # enabling concurrent S matmuls on both PE row halves.
            qT2 = big.tile([2 * C, HPC, T], mm_dt)
            kT2 = big.tile([2 * C, HPC, (NB // 2) * P], mm_dt)
            v_sb = big.tile([P, HPC, NB, D + 1], av_dt)
            ones_col = const.tile([P, 1], F32)
            nc.vector.memset(ones_col[:, :], 1.0)
            nc.vector.tensor_copy(
                v_sb[:, :, :, D : D + 1],
                ones_col[:, :].to_broadcast((P, HPC, NB, 1)),
            )
            oT_sb = big.tile([C, HPC, T], mm_dt)
            SPP = 32 * (NCH - 1) + 1
            spacks = [big.tile([SPP, TCH], F32, name=f"spack{h}") for h in range(HPC)]
            for h in range(HPC):
                # chunk-granular: produce q/k/v for chunk c, then immediately
                # emit attention for chunk c (its s-range is exactly covered
                # by chunks <= c), maximizing pipeline overlap.
                for c in range(NCH):
                    n4 = c
                    sl = slice(n4 * TCH, (n4 + 1) * TCH)
                    qp = ps_m.tile([C, TCH], F32, tag="mps")
                    nc.tensor.matmul(
                        qp[:, :], lhsT=wq_sb[:, h, :],
                        rhs=xT[:, sl], start=True, stop=True,
                    )
                    nc.vector.tensor_copy(qT2[0:C, h, sl], qp[:, :])
                    nc.vector.tensor_copy(qT2[C : 2 * C, h, sl], qp[:, :])
                    kp = ps_m.tile([C, TCH], F32, tag="mps")
                    nc.tensor.matmul(
                        kp[:, :], lhsT=wk_sb[:, h, :],
                        rhs=yT[:, sl], start=True, stop=True,
                    )
                    kpv = kp[:, :].rearrange("p (a b) -> p a b", b=P)
                    kdst = kT2[:, h, 2 * n4 * P : (2 * n4 + 2) * P].rearrange(
                        "p (a b) -> p a b", b=P
                    )
                    nc.vector.tensor_copy(kdst[0:C], kpv[:, 0:4:2, :])
                    nc.vector.tensor_copy(kdst[C : 2 * C], kpv[:, 1:4:2, :])
                    vp = ps_m.tile([P, BPC * D], F32, tag="mps")
                    for jj in range(BPC):
                        j = BPC * c + jj
                        nc.tensor.matmul(
                            vp[:, jj * D : (jj + 1) * D],
                            lhsT=yT[:, j * P : (j + 1) * P],
                            rhs=wv_sb[:, h, :], start=True, stop=True,
                        )
                    nc.vector.tensor_copy(
                        v_sb[:, h, BPC * c : BPC * (c + 1), 0:D],
                        vp[:, :].rearrange("p (a b) -> p a b", b=D),
                    )

                    # ---- attention for chunk c --------------------------
                    ns = BPC * c + BPC
                    ops = ps_o.tile([D + 1, TCH], F32, tag="oT")
                    for pr in range(ns // 2):
                        jA, jB = 2 * pr, 2 * pr + 1
                        sub = []
                        for half, j in ((0, jA), (1, jB)):
                            t0 = max(0, P * j - TCH * c)
                            sps = ps_s.tile([P, TCH], F32, tag="S", name=f"sps{half}")
                            nc.tensor.matmul(
                                sps[:, t0:TCH],
                                lhsT=kT2[half * C : (half + 1) * C, h,
                                         pr * P : (pr + 1) * P],
                                rhs=qT2[half * C : (half + 1) * C, h,
                                        c * TCH + t0 : (c + 1) * TCH],
                                start=True, stop=True,
                            )
                            sub.append((j, t0, sps))
                        for j, t0, sps in sub:
                            if j >= BPC * c:  # diagonal tile: mask the triangle
                                nc.vector.tensor_add(
                                    sps[:, t0 : t0 + P], sps[:, t0 : t0 + P], tri[:, :]
                                )
                            es = work.tile([P, TCH], av_dt, tag="expS")
                            nc.scalar.activation(
                                es[:, t0:TCH], sps[:, t0:TCH],
                                mybir.ActivationFunctionType.Exp, scale=scale,
                            )
                            nc.tensor.matmul(
                                ops[:, t0:TCH],
                                lhsT=v_sb[:, h, j, :],
                                rhs=es[:, t0:TCH],
                                start=(j == 0), stop=(j == ns - 1),
                                skip_group_check=True,
                            )
                    nc.vector.tensor_copy(
                        oT_sb[:, h, c * TCH : (c + 1) * TCH], ops[0:C, :]
                    )
                    nc.vector.tensor_copy(
                        spacks[h][32 * c : 32 * c + 1, :], ops[D : D + 1, :]
                    )

            # ---- reciprocal of softmax sums -------------------------------
            # transpose each head's [97, 512] sums pack (valid rows 32c) into
            # [128, 97] psum blocks; reciprocal only the valid strided columns.
            recip = big.tile([P, HPC, BPC, NCH], F32)
            for h in range(HPC):
                rp = ps_s.tile([P, TCH], F32, tag="S", name=f"rps{h}")
                for sc in range(BPC):
                    nc.tensor.transpose(
                        rp[:, sc * P : sc * P + SPP],
                        spacks[h][0:SPP, sc * P : (sc + 1) * P],
                        ident[0:SPP, 0:SPP],
                    )
                rp_valid = rp[:, :].rearrange("p (a b) -> p a b", a=BPC)[:, :, 0 : 32 * NCH : 32]
                nc.vector.reciprocal(recip[:, h, :, :], rp_valid)

            # ---- projection + normalization + bias ------------------------
            out_full = big.tile([P, NSIG, C], F32)
            for sg in range(NSIG):
                c, sc = sg // BPC, sg % BPC
                pp = ps_m.tile([P, HPC * C], F32, tag="mps")
                for h in range(HPC):
                    nc.tensor.matmul(
                        pp[:, h * C : (h + 1) * C],
                        lhsT=oT_sb[:, h, sg * P : (sg + 1) * P],
                        rhs=wp_sb[:, h, :], start=True, stop=True,
                    )
                acc = bias[:, :]
                for h in range(HPC):
                    dst = (
                        out_full[:, sg, :]
                        if h == HPC - 1
                        else work.tile([P, C], F32, tag="acc", name="acc")[:, :]
                    )
                    nc.vector.scalar_tensor_tensor(
                        out=dst,
                        in0=pp[:, h * C : (h + 1) * C],
                        scalar=recip[:, h, sc, c : c + 1],
                        in1=acc,
                        op0=mybir.AluOpType.mult,
                        op1=mybir.AluOpType.add,
                    )
                    acc = dst
            outv = out_d[:].rearrange("(n p) c -> p n c", p=P)
            for g in range(4):
                sl = slice(g * (NSIG // 4), (g + 1) * (NSIG // 4))
                nc.sync.dma_start(out=outv[:, sl, :], in_=out_full[:, sl, :])

    nc.compile()
    return nc


_prog = None


def _get_program():
    global _prog
    if _prog is None:
        _prog = build_program()
    return _prog


def make_host_consts():
    i = np.arange(P)
    # tri[s_rel, t_rel]: valid (0) when t_rel >= s_rel else -1e30
    tri = np.where(i[None, :] >= i[:, None], 0.0, NEG).astype(np.float32)
    ident = np.eye(P, dtype=np.float32)
    return tri, ident


def make_in_maps(inputs=None, **kw):
    if inputs is None:
        inputs = kw
    x = np.ascontiguousarray(np.asarray(inputs["x"], np.float32))
    y = np.ascontiguousarray(np.asarray(inputs["y"], np.float32))
    Wq = np.ascontiguousarray(np.asarray(inputs["Wq"], np.float32))
    Wk = np.ascontiguousarray(np.asarray(inputs["Wk"], np.float32))
    Wv = np.ascontiguousarray(np.asarray(inputs["Wv"], np.float32))
    Wp = np.ascontiguousarray(np.asarray(inputs["W_proj"], np.float32)).reshape(
        H, D, C
    )
    b_proj = np.asarray(inputs["b_proj"], np.float32)
    tri, ident = make_host_consts()
    zeros_c = np.zeros_like(b_proj)

    in_maps = []
    for core in range(NCORES):
        b, half = core // 2, core % 2
        h0 = HPC * half
        in_maps.append(
            {
                "x_loc": x[b],
                "y_loc": y[b],
                "wq_loc": Wq[h0 : h0 + HPC],
                "wk_loc": Wk[h0 : h0 + HPC],
                "wv_loc": Wv[h0 : h0 + HPC],
                "wp_loc": Wp[h0 : h0 + HPC],
                "bias_loc": (b_proj if half == 0 else zeros_c)[None, :],
                "tri_loc": tri,
                "ident_loc": ident,
            }
        )
    return in_maps


def assemble(results):
    out = np.stack(
        [results[2 * b]["out_loc"] + results[2 * b + 1]["out_loc"] for b in range(B)]
    )
    return out.astype(np.float32)


def kernel(x, y, Wq, Wk, Wv, W_proj, b_proj):
    in_maps = make_in_maps(
        x=x, y=y, Wq=Wq, Wk=Wk, Wv=Wv, W_proj=W_proj, b_proj=b_proj
    )
    nc = _get_program()
    res = run_bass_kernel_spmd(nc, in_maps, list(range(NCORES))).results
    return assemble(res)
